# revision 36
# baseline (speedup 1.0000x reference)
"""MoE FFN (8 experts, top-2, dense-combine reference) on 8 Trainium2 cores.

Strategy (expert-parallel, per the sharding hint):
  - Each core c owns expert c: computes y_c = wsel_c(t) * (GELU(x @ W1_c^T + b1_c) @ W2_c^T + b2_c)
    for ALL tokens, where wsel_c(t) is the renormalized top-2 gate weight of
    expert c for token t (0 if expert c not in token t's top-2).
  - The gate (fp32 logits -> softmax -> top-2 renormalize) is replicated on
    every core and computed on-device in fp32 (top-2 selection is
    numerically delicate: min top2/top3 prob gap is ~3e-5).
  - Host combine: out = sum_c y_c  (the expert-parallel "reduce"), gate_probs
    comes from core 0.

Layouts (host pre-transposes so the contraction dim lands on partitions):
  xT32 [D, N] fp32, xTbf [D, N] bf16, gwT [D, E] fp32, w1T [D, F] bf16,
  w2T [F, D] bf16, b1r [128, F/128] fp32 (b1r[p, fc] = b1[fc*128+p]),
  b2big [128, D] fp32 (b2 row broadcast), esel [128, E] one-hot of expert c.

Device outputs: y [N, D] fp32; gp_raw [128, (N/128)*E] fp32 packed probs
  (gp_raw[p, t128*E + e] = prob(token t128*128 + p, e)).
"""

import numpy as np
import ml_dtypes

import concourse.bacc as bacc
import concourse.mybir as mybir
import concourse.tile as tile
from concourse import bass_utils

F32 = mybir.dt.float32
BF16 = mybir.dt.bfloat16
AF = mybir.ActivationFunctionType
ALU = mybir.AluOpType
AX = mybir.AxisListType

D_MODEL, D_FF, N_EXPERTS, TOP_K = 1024, 4096, 8, 2
N_TOKENS = 4096
N_CORES = 8
T_BLK = 512


def build_nc(n_tok=N_TOKENS, d=D_MODEL, f=D_FF, e=N_EXPERTS, t_blk=T_BLK,
             act=AF.Gelu):
    KC = d // 128        # contraction chunks over d
    TC = n_tok // 128    # token 128-chunks
    FC = f // 128        # hidden-dim 128-chunks
    NTB = n_tok // t_blk # token blocks
    TCB = t_blk // 128   # 128-token chunks per block
    FG = min(512, f)     # w1 streaming group width along f
    NFG = f // FG
    FS = FG // 128

    nc = bacc.Bacc("TRN2", target_bir_lowering=False, debug=False,
                   num_devices=N_CORES)

    xT32 = nc.dram_tensor("xT32", [d, n_tok], F32, kind="ExternalInput")
    xTbf = nc.dram_tensor("xTbf", [d, n_tok], BF16, kind="ExternalInput")
    gwT = nc.dram_tensor("gwT", [d, e], F32, kind="ExternalInput")
    w1T = nc.dram_tensor("w1T", [d, f], BF16, kind="ExternalInput")
    w2T = nc.dram_tensor("w2T", [f, d], BF16, kind="ExternalInput")
    b1r = nc.dram_tensor("b1r", [128, FC], F32, kind="ExternalInput")
    b2big = nc.dram_tensor("b2big", [128, d], F32, kind="ExternalInput")
    esel = nc.dram_tensor("esel", [128, e], F32, kind="ExternalInput")
    y = nc.dram_tensor("y", [n_tok, d], F32, kind="ExternalOutput")
    gpr = nc.dram_tensor("gp_raw", [128, TC * e], F32, kind="ExternalOutput")

    xTbf_r = xTbf.ap().rearrange("(kc p) t -> p kc t", p=128)
    w1T_r = w1T.ap().rearrange("(kc p) ff -> p kc ff", p=128)
    w2T_r = w2T.ap().rearrange("(fc p) dd -> p fc dd", p=128)
    gwT_r = gwT.ap().rearrange("(kc p) ee -> p kc ee", p=128)

    with tile.TileContext(nc) as tc:
        with (
            tc.tile_pool(name="persist", bufs=1) as persist,
            tc.tile_pool(name="xblk", bufs=2) as xpool,
            tc.tile_pool(name="hblk", bufs=1) as hpool,
            tc.tile_pool(name="w1s", bufs=2) as w1pool,
            tc.tile_pool(name="ps1", bufs=2, space="PSUM") as ps1pool,
            tc.tile_pool(name="ps2", bufs=2, space="PSUM") as ps2pool,
            tc.tile_pool(name="yev", bufs=3) as ypool,
        ):
            gw_sb = persist.tile([128, KC, e], F32)
            nc.sync.dma_start(gw_sb[:], gwT_r)
            b1_sb = persist.tile([128, FC], F32)
            nc.sync.dma_start(b1_sb[:], b1r.ap())
            b2_sb = persist.tile([128, d], F32)
            nc.sync.dma_start(b2_sb[:], b2big.ap())
            esel_sb = persist.tile([128, e], F32)
            nc.sync.dma_start(esel_sb[:], esel.ap())
            w2sb = persist.tile([128, FC, d], BF16)
            nc.sync.dma_start(w2sb[:], w2T_r)
            # gate state
            exp3 = persist.tile([128, TC, e], F32)
            probs3 = persist.tile([128, TC, e], F32)
            wsel = persist.tile([128, TC], F32)

            def emit_mm1(tb):
                """h^T for token block tb: htb[p, fc, t] = gelu(x @ W1^T + b1)."""
                xtb = xpool.tile([128, KC, t_blk], BF16)
                nc.sync.dma_start(
                    xtb[:], xTbf_r[:, :, tb * t_blk:(tb + 1) * t_blk])
                htb = hpool.tile([128, FC, t_blk], BF16)
                for fg in range(NFG):
                    w1g = w1pool.tile([128, KC, FG], BF16)
                    nc.sync.dma_start(
                        w1g[:], w1T_r[:, :, fg * FG:(fg + 1) * FG])
                    for fs in range(FS):
                        fc = fg * FS + fs
                        ps = ps1pool.tile([128, t_blk], F32)
                        for kc in range(KC):
                            nc.tensor.matmul(
                                ps[:],
                                w1g[:, kc, fs * 128:(fs + 1) * 128],
                                xtb[:, kc, :],
                                start=(kc == 0), stop=(kc == KC - 1))
                        nc.scalar.activation(
                            htb[:, fc, :], ps[:], act,
                            bias=b1_sb[:, fc:fc + 1], scale=1.0)
                return htb

            def emit_gate():
                with (
                    tc.tile_pool(name="gx", bufs=4) as gxp,
                    tc.tile_pool(name="gps", bufs=2, space="PSUM") as gpp,
                    tc.tile_pool(name="gtmp", bufs=1) as gtp,
                ):
                    for t in range(TC):
                        ps = gpp.tile([128, e], F32)
                        for kc in range(KC):
                            xt = gxp.tile([128, 128], F32)
                            nc.sync.dma_start(
                                xt[:],
                                xT32.ap()[kc * 128:(kc + 1) * 128,
                                          t * 128:(t + 1) * 128])
                            nc.tensor.matmul(
                                ps[:], xt[:], gw_sb[:, kc, :],
                                start=(kc == 0), stop=(kc == KC - 1))
                        nc.vector.tensor_copy(exp3[:, t, :], ps[:])
                    # softmax over e (innermost)
                    mx = gtp.tile([128, TC], F32)
                    nc.vector.tensor_reduce(mx[:], exp3[:], AX.X, ALU.max)
                    nc.vector.tensor_tensor(
                        exp3[:], exp3[:], mx[:].to_broadcast([128, TC, e]),
                        ALU.subtract)
                    nc.scalar.activation(exp3[:], exp3[:], AF.Exp)
                    s = gtp.tile([128, TC], F32)
                    nc.vector.tensor_reduce(s[:], exp3[:], AX.X, ALU.add)
                    rs = gtp.tile([128, TC], F32)
                    nc.vector.reciprocal(rs[:], s[:])
                    nc.vector.tensor_tensor(
                        probs3[:], exp3[:], rs[:].to_broadcast([128, TC, e]),
                        ALU.mult)
                    nc.sync.dma_start(gpr.ap(), probs3[:])
                    # top-2 over e: m1e = max(exp), zap, m2e = 2nd max
                    m1e = gtp.tile([128, TC], F32)
                    nc.vector.tensor_reduce(m1e[:], exp3[:], AX.X, ALU.max)
                    eq1 = gtp.tile([128, TC, e], F32)
                    nc.vector.tensor_tensor(
                        eq1[:], exp3[:], m1e[:].to_broadcast([128, TC, e]),
                        ALU.is_ge)
                    t1 = gtp.tile([128, TC, e], F32)
                    nc.vector.tensor_tensor(t1[:], eq1[:], exp3[:], ALU.mult)
                    nc.vector.tensor_tensor(t1[:], exp3[:], t1[:],
                                            ALU.subtract)
                    m2e = gtp.tile([128, TC], F32)
                    nc.vector.tensor_reduce(m2e[:], t1[:], AX.X, ALU.max)
                    # top-2 mask, this core's expert, renormalize:
                    # wsel = exp * (exp >= m2e) . esel / (1 + m2e)
                    ge2 = gtp.tile([128, TC, e], F32)
                    nc.vector.tensor_tensor(
                        ge2[:], exp3[:], m2e[:].to_broadcast([128, TC, e]),
                        ALU.is_ge)
                    nc.vector.tensor_tensor(ge2[:], ge2[:], exp3[:], ALU.mult)
                    nc.vector.tensor_tensor(
                        ge2[:], ge2[:],
                        esel_sb[:, None, :].to_broadcast([128, TC, e]),
                        ALU.mult)
                    nc.vector.tensor_reduce(wsel[:], ge2[:], AX.X, ALU.add)
                    m2p1 = gtp.tile([128, TC], F32)
                    nc.vector.tensor_scalar_add(m2p1[:], m2e[:], 1.0)
                    winv = gtp.tile([128, TC], F32)
                    nc.vector.reciprocal(winv[:], m2p1[:])
                    nc.vector.tensor_tensor(wsel[:], wsel[:], winv[:],
                                            ALU.mult)

            def emit_mm2(tb, htb):
                """y rows for block tb: y = ((h @ W2^T) + b2) * wsel."""
                for tcb in range(TCB):
                    tg = tb * TCB + tcb
                    ps2 = ps2pool.tile([128, d], F32)
                    dw = min(512, d)
                    for fc in range(FC):
                        lhsT = htb[:, fc, tcb * 128:(tcb + 1) * 128]
                        for dh in range(d // dw):
                            nc.tensor.matmul(
                                ps2[:, dh * dw:(dh + 1) * dw],
                                lhsT,
                                w2sb[:, fc, dh * dw:(dh + 1) * dw],
                                start=(fc == 0), stop=(fc == FC - 1))
                    yb = ypool.tile([128, d], F32)
                    nc.vector.tensor_add(yb[:], ps2[:], b2_sb[:])
                    nc.vector.tensor_scalar_mul(yb[:], yb[:],
                                                wsel[:, tg:tg + 1])
                    nc.sync.dma_start(
                        y.ap()[tg * 128:(tg + 1) * 128, :], yb[:])

            htb0 = emit_mm1(0)
            emit_gate()
            emit_mm2(0, htb0)
            for tb in range(1, NTB):
                htb = emit_mm1(tb)
                emit_mm2(tb, htb)

    nc.compile()
    return nc


def build_nc_sparse(n_tok=N_TOKENS, d=D_MODEL, f=D_FF, e=N_EXPERTS,
                    cap=1152, act=AF.Gelu, stage=5):
    """Sparse expert-parallel: each core gathers only the tokens routed to
    its expert (top-2 of the gate), computes the FFN on those, and writes
    compacted rows + their token ids; host scatter-adds."""
    import concourse.bass_isa as bass_isa  # noqa: F401
    I32 = mybir.dt.int32
    I16 = mybir.dt.int16
    U32 = mybir.dt.uint32

    KC = d // 128
    TC = n_tok // 128
    FC = f // 128
    NW = n_tok // 16        # wrapped free size for sparse_gather input
    CW = cap // 16          # wrapped free size for compacted lists
    CPC = cap // 128        # 128-token chunks of capacity
    NTG = n_tok // 512      # gate t-groups

    assert cap % 128 == 0 and CW <= 512

    nc = bacc.Bacc("TRN2", target_bir_lowering=False, debug=False,
                   num_devices=N_CORES)

    xThi = nc.dram_tensor("xThi", [d, n_tok], BF16, kind="ExternalInput")
    xTlo = nc.dram_tensor("xTlo", [d, n_tok], BF16, kind="ExternalInput")
    xrow = nc.dram_tensor("xrow", [n_tok, d], BF16, kind="ExternalInput")
    gwThi = nc.dram_tensor("gwThi", [d, e], BF16, kind="ExternalInput")
    gwTlo = nc.dram_tensor("gwTlo", [d, e], BF16, kind="ExternalInput")
    w1T = nc.dram_tensor("w1T", [d, f], BF16, kind="ExternalInput")
    w2T = nc.dram_tensor("w2T", [f, d], BF16, kind="ExternalInput")
    b1r = nc.dram_tensor("b1r", [128, FC], F32, kind="ExternalInput")
    b2big = nc.dram_tensor("b2big", [128, d], F32, kind="ExternalInput")
    esel = nc.dram_tensor("esel", [128, e], F32, kind="ExternalInput")
    ids_in = nc.dram_tensor("ids_in", [16, NW], F32, kind="ExternalInput")
    yg = nc.dram_tensor("yg", [cap, d], F32, kind="ExternalOutput")
    ids_out = nc.dram_tensor("ids_out", [16, CW], I32, kind="ExternalOutput")
    cnt_out = nc.dram_tensor("cnt_out", [1, 1], U32, kind="ExternalOutput")
    gpr = nc.dram_tensor("gp_raw", [128, TC * e], F32, kind="ExternalOutput")

    xThi_r = xThi.ap().rearrange("(kc p) t -> p kc t", p=128)
    xTlo_r = xTlo.ap().rearrange("(kc p) t -> p kc t", p=128)
    w1T_r = w1T.ap().rearrange("(kc p) ff -> p kc ff", p=128)
    w2T_r = w2T.ap().rearrange("(fc p) dd -> p fc dd", p=128)
    gwThi_r = gwThi.ap().rearrange("(kc p) ee -> p kc ee", p=128)
    gwTlo_r = gwTlo.ap().rearrange("(kc p) ee -> p kc ee", p=128)

    from concourse import library_config

    def _emit(tc):
        with (
            tc.tile_pool(name="persist", bufs=1) as persist,
            tc.tile_pool(name="route", bufs=1) as route,
        ):
            # GpSimd ucode-library switches drain in-flight SWDGE DMA and
            # reload Q7 IRAM (~5-25us). Preload the sparse_gather library at
            # kernel start so its switch runs under the gate phase instead of
            # on the routing critical path.
            nc.gpsimd.load_library(library_config.sparse_gather)
            gwhi_sb = persist.tile([128, KC, e], BF16)
            nc.sync.dma_start(gwhi_sb[:], gwThi_r)
            gwlo_sb = persist.tile([128, KC, e], BF16)
            nc.sync.dma_start(gwlo_sb[:], gwTlo_r)
            b1_sb = persist.tile([128, FC], F32)
            nc.sync.dma_start(b1_sb[:], b1r.ap())
            b2_sb = persist.tile([128, d], F32)
            nc.sync.dma_start(b2_sb[:], b2big.ap())
            esel_sb = persist.tile([128, e], F32)
            nc.sync.dma_start(esel_sb[:], esel.ap())
            w2sb = persist.tile([128, FC, d], BF16)
            exp3 = persist.tile([128, TC, e], F32)
            probs3 = persist.tile([128, TC, e], F32)
            wsel = persist.tile([128, TC], F32)

            # ---------------- gate ----------------
            # processed per 512-token group; the softmax/top-2 chain for
            # group g runs on DVE/ACT while the PE computes group g+1
            with (
                tc.tile_pool(name="gx", bufs=3) as gxp,
                tc.tile_pool(name="gps", bufs=4, space="PSUM") as gpp,
                tc.tile_pool(name="gtmp", bufs=2) as gtp,
            ):
                for tg in range(NTG):
                    # split-bf16 gate: logits = xhi@ghi + xlo@ghi + xhi@glo
                    # (~2^-16 relative accuracy; dropped xlo@glo is ~2^-24)
                    gxh = gxp.tile([128, KC, 512], BF16, tag="gxh")
                    nc.sync.dma_start(
                        gxh[:], xThi_r[:, :, tg * 512:(tg + 1) * 512])
                    gxl = gxp.tile([128, KC, 512], BF16, tag="gxl")
                    nc.sync.dma_start(
                        gxl[:], xTlo_r[:, :, tg * 512:(tg + 1) * 512])
                    for ts4 in range(4):
                        t = tg * 4 + ts4
                        ps = gpp.tile([128, e], F32)
                        for kc in range(KC):
                            xh = gxh[:, kc, ts4 * 128:(ts4 + 1) * 128]
                            xl = gxl[:, kc, ts4 * 128:(ts4 + 1) * 128]
                            nc.tensor.matmul(
                                ps[:], xh, gwhi_sb[:, kc, :],
                                start=(kc == 0), stop=False)
                            nc.tensor.matmul(
                                ps[:], xh, gwlo_sb[:, kc, :],
                                start=False, stop=False)
                            nc.tensor.matmul(
                                ps[:], xl, gwhi_sb[:, kc, :],
                                start=False, stop=(kc == KC - 1))
                        nc.vector.tensor_copy(exp3[:, t, :], ps[:])
                    # softmax + top-2 for this group's token slice
                    TG4 = 4
                    sl = slice(tg * TG4, (tg + 1) * TG4)
                    E3 = exp3[:, sl, :]
                    P3 = probs3[:, sl, :]
                    mx = gtp.tile([128, TG4], F32, tag="mx")
                    nc.vector.tensor_reduce(mx[:], E3, AX.X, ALU.max)
                    nc.vector.tensor_tensor(
                        E3, E3, mx[:].to_broadcast([128, TG4, e]),
                        ALU.subtract)
                    nc.scalar.activation(E3, E3, AF.Exp)
                    s = gtp.tile([128, TG4], F32, tag="s")
                    nc.vector.tensor_reduce(s[:], E3, AX.X, ALU.add)
                    rs = gtp.tile([128, TG4], F32, tag="rs")
                    nc.vector.reciprocal(rs[:], s[:])
                    nc.vector.tensor_tensor(
                        P3, E3, rs[:].to_broadcast([128, TG4, e]), ALU.mult)
                    m1e = gtp.tile([128, TG4], F32, tag="m1e")
                    nc.vector.tensor_reduce(m1e[:], E3, AX.X, ALU.max)
                    eq1 = gtp.tile([128, TG4, e], F32, tag="eq1")
                    nc.vector.tensor_tensor(
                        eq1[:], E3, m1e[:].to_broadcast([128, TG4, e]),
                        ALU.is_ge)
                    t1 = gtp.tile([128, TG4, e], F32, tag="t1")
                    nc.vector.tensor_tensor(t1[:], eq1[:], E3, ALU.mult)
                    nc.vector.tensor_tensor(t1[:], E3, t1[:], ALU.subtract)
                    m2e = gtp.tile([128, TG4], F32, tag="m2e")
                    nc.vector.tensor_reduce(m2e[:], t1[:], AX.X, ALU.max)
                    ge2 = gtp.tile([128, TG4, e], F32, tag="ge2")
                    nc.vector.tensor_tensor(
                        ge2[:], E3, m2e[:].to_broadcast([128, TG4, e]),
                        ALU.is_ge)
                    nc.vector.tensor_tensor(ge2[:], ge2[:], E3, ALU.mult)
                    nc.vector.tensor_tensor(
                        ge2[:], ge2[:],
                        esel_sb[:, None, :].to_broadcast([128, TG4, e]),
                        ALU.mult)
                    nc.vector.tensor_reduce(wsel[:, sl], ge2[:], AX.X,
                                            ALU.add)
                    m2p1 = gtp.tile([128, TG4], F32, tag="m2p1")
                    nc.vector.tensor_scalar_add(m2p1[:], m2e[:], 1.0)
                    winv = gtp.tile([128, TG4], F32, tag="winv")
                    nc.vector.reciprocal(winv[:], m2p1[:])
                    nc.vector.tensor_tensor(wsel[:, sl], wsel[:, sl],
                                            winv[:], ALU.mult)
                nc.sync.dma_start(gpr.ap(), probs3[:])
            if stage <= 1:
                return

            # ---------------- routing: compact this expert's tokens --------
            # wrapped layout: token t at [t % 16, t // 16]
            wselw = route.tile([16, NW], F32)
            wselw3 = wselw[:].rearrange("q (ft a) -> q ft a", a=8)
            for a in range(8):
                # partition-base-16a access: engines need quadrant-aligned
                # partition starts, so shuffle via DMA
                nc.sync.dma_start(
                    wselw3[:, :, a], wsel[16 * a:16 * (a + 1), :])
            # wrapped token ids [16, NW] as f32, provided by the host (a
            # static iota; avoids a GpSimd library switch for InstIota)
            ids_f = route.tile([16, NW], F32)
            nc.sync.dma_start(ids_f[:], ids_in.ap())
            maskw = route.tile([16, NW], F32)
            nc.vector.tensor_scalar(maskw[:], wselw[:], 0.0, None,
                                    op0=ALU.is_gt)
            # sel_id = (id+1)*mask - 1  (id where selected, -1 elsewhere)
            sel_id = route.tile([16, NW], F32)
            nc.vector.tensor_scalar(sel_id[:], ids_f[:], 1.0, None,
                                    op0=ALU.add)
            nc.vector.tensor_tensor(sel_id[:], sel_id[:], maskw[:], ALU.mult)
            nc.vector.tensor_scalar(sel_id[:], sel_id[:], 1.0, None,
                                    op0=ALU.subtract)
            # sel_w = w + (mask-1)  (w>0 where selected, -1 elsewhere)
            sel_w = route.tile([16, NW], F32)
            m1t = route.tile([16, NW], F32)
            nc.vector.tensor_scalar(m1t[:], maskw[:], 1.0, None,
                                    op0=ALU.subtract)
            nc.vector.tensor_add(sel_w[:], wselw[:], m1t[:])

            ids_c = route.tile([16, CW], F32)
            cnt = route.tile([1, 1], U32)
            nc.vector.memset(ids_c[:], -1.0)
            nc.gpsimd.sparse_gather(ids_c[:], sel_id[:], num_found=cnt[:])
            w_c = route.tile([16, CW], F32)
            cnt2 = route.tile([1, 1], U32)
            nc.vector.memset(w_c[:], -1.0)
            nc.gpsimd.sparse_gather(w_c[:], sel_w[:], num_found=cnt2[:])
            # switch to the dma_gather library NOW so the reload overlaps the
            # tail-fix / index-shuffle work below instead of serializing
            # right before the gather descriptor generation
            nc.gpsimd.load_library(library_config.mlp)
            nc.sync.dma_start(cnt_out.ap(), cnt[:])
            # On HW the tail beyond num_found is uninitialized garbage (the
            # sim fills -1): force tail slots to 0 by position, overwriting
            # whatever junk is there (ids -> token 0, weights -> 0).
            # cnt is broadcast to 16 partitions with a K=1 ones matmul on the
            # (idle) PE instead of gpsimd.partition_broadcast — keeps the
            # GpSimd op sequence inside a single ucode library.
            cnt_f = route.tile([1, 1], F32)
            nc.vector.tensor_copy(cnt_f[:], cnt[:])
            ones16 = route.tile([1, 16], F32)
            nc.vector.memset(ones16[:], 1.0)
            with tc.tile_pool(name="cps", bufs=1, space="PSUM") as cpsp:
                cps = cpsp.tile([16, 1], F32)
                nc.tensor.matmul(cps[:], ones16[:], cnt_f[:],
                                 start=True, stop=True)
                cntb = route.tile([16, 1], F32)
                nc.vector.tensor_copy(cntb[:], cps[:])
            notkeep = route.tile([16, CW], U32)
            nc.vector.tensor_tensor(notkeep[:], ids_f[:, :CW],
                                    cntb[:].to_broadcast([16, CW]), ALU.is_ge)
            zeros16 = route.tile([16, CW], F32)
            nc.vector.memset(zeros16[:], 0.0)
            nc.vector.copy_predicated(ids_c[:], notkeep[:], zeros16[:])
            nc.vector.copy_predicated(w_c[:], notkeep[:], zeros16[:])
            ids32 = route.tile([16, CW], I32)
            nc.vector.tensor_copy(ids32[:], ids_c[:])
            nc.sync.dma_start(ids_out.ap(), ids32[:])
            ids16 = route.tile([16, CW], I16)
            nc.vector.tensor_copy(ids16[:], ids_c[:])
            idx128 = route.tile([128, CW], I16)
            for a in range(8):
                nc.sync.dma_start(idx128[16 * a:16 * (a + 1), :], ids16[:])
            # per-128-chunk gate weights: wpart[16a+q, c] = w_c[q, 8c+a]
            wpart = route.tile([128, CPC], F32)
            w_c3 = w_c[:].rearrange("q (c a) -> q c a", a=8)
            for a in range(8):
                nc.sync.dma_start(wpart[16 * a:16 * (a + 1), :],
                                  w_c3[:, :, a])

            if stage <= 2:
                return

            # ---------------- gather x rows ----------------
            with (
                tc.tile_pool(name="xg", bufs=1) as xgp,
                tc.tile_pool(name="hg", bufs=1) as hgp,
                tc.tile_pool(name="w1s", bufs=2) as w1pool,
                tc.tile_pool(name="ps1", bufs=2, space="PSUM") as ps1pool,
                tc.tile_pool(name="ps2", bufs=2, space="PSUM") as ps2pool,
                tc.tile_pool(name="yev", bufs=3) as ypool,
            ):
                # chunks of the capacity: <=512 wide for the matmul free-dim
                # limit AND for dma_gather (one gather's s2m descriptor
                # count must fit the 128-entry SWDGE ring -> <=512 idxs)
                chunks = []
                off = 0
                while off < cap:
                    cwid = min(512, cap - off)
                    chunks.append((off, cwid))
                    off += cwid

                xg_tiles = {}
                for (off, cwid) in chunks:
                    xgc = xgp.tile([128, KC, cwid], BF16, tag=f"xg{off}")
                    nc.gpsimd.dma_gather(
                        xgc[:], xrow.ap(),
                        idx128[:, off // 16:(off + cwid) // 16],
                        num_idxs=cwid, num_idxs_reg=cwid, elem_size=d,
                        transpose=True)
                    xg_tiles[off] = xgc
                # w2 (8MB) is first needed by mm2. The GpSimd ucode-library
                # switch before the gathers drains ALL in-flight SWDGE DMA,
                # so an early-running w2 transfer would stall it ~22us.
                # Write a dummy sliver of w2sb from the last gather's output
                # first: the WAW dependency forces the w2 DMA after the
                # gathers have issued.
                last_xg = xg_tiles[chunks[-1][0]]
                nc.vector.tensor_copy(w2sb[:, 0, 0:2], last_xg[:, 0, 0:2])
                nc.sync.dma_start(w2sb[:], w2T_r)
                if stage <= 3:
                    ytmp = ypool.tile([128, d], F32)
                    nc.vector.tensor_copy(ytmp[:, 0:cap // 4],
                                          xg_tiles[0][:, 0, 0:cap // 4])
                    nc.sync.dma_start(yg.ap()[0:128, :], ytmp[:])
                    return

                hg = hgp.tile([128, FC, cap], BF16)
                FG = min(512, f)
                for fg in range(f // FG):
                    w1g = w1pool.tile([128, KC, FG], BF16)
                    nc.sync.dma_start(
                        w1g[:], w1T_r[:, :, fg * FG:(fg + 1) * FG])
                    for fs in range(FG // 128):
                        fc = fg * (FG // 128) + fs
                        for (off, cwid) in chunks:
                            ps = ps1pool.tile([128, 512], F32)
                            for kc in range(KC):
                                nc.tensor.matmul(
                                    ps[:, :cwid],
                                    w1g[:, kc, fs * 128:(fs + 1) * 128],
                                    xg_tiles[off][:, kc, :],
                                    start=(kc == 0), stop=(kc == KC - 1))
                            nc.scalar.activation(
                                hg[:, fc, off:off + cwid], ps[:, :cwid],
                                act, bias=b1_sb[:, fc:fc + 1], scale=1.0)

                if stage <= 4:
                    ytmp = ypool.tile([128, d], F32)
                    nc.vector.tensor_copy(ytmp[:, 0:cap // 4],
                                          hg[:, 0, 0:cap // 4])
                    nc.sync.dma_start(yg.ap()[0:128, :], ytmp[:])
                    return

                dw = min(512, d)
                for tcb in range(CPC):
                    ps2 = ps2pool.tile([128, d], F32)
                    for fc in range(FC):
                        lhsT = hg[:, fc, tcb * 128:(tcb + 1) * 128]
                        for dh in range(d // dw):
                            nc.tensor.matmul(
                                ps2[:, dh * dw:(dh + 1) * dw],
                                lhsT,
                                w2sb[:, fc, dh * dw:(dh + 1) * dw],
                                start=(fc == 0), stop=(fc == FC - 1))
                    yb = ypool.tile([128, d], F32)
                    nc.vector.tensor_add(yb[:], ps2[:], b2_sb[:])
                    nc.vector.tensor_scalar_mul(yb[:], yb[:],
                                                wpart[:, tcb:tcb + 1])
                    nc.sync.dma_start(
                        yg.ap()[tcb * 128:(tcb + 1) * 128, :], yb[:])

    with tile.TileContext(nc) as tc:
        _emit(tc)
    nc.compile()
    return nc


_NC_CACHE = {}


def _get_nc():
    key = (N_TOKENS, D_MODEL, D_FF, N_EXPERTS, T_BLK)
    if key not in _NC_CACHE:
        _NC_CACHE[key] = build_nc(*key)
    return _NC_CACHE[key]


def prep_core_inputs(x, gate_w, w1, b1, w2, b2, n_cores=N_CORES):
    """Host-side sharding: per-core input dicts (expert-parallel)."""
    bf16 = ml_dtypes.bfloat16
    n = x.shape[0] * x.shape[1]
    d = x.shape[2]
    f = w1.shape[1]
    xT32 = np.ascontiguousarray(x.reshape(n, d).T.astype(np.float32))
    xTbf = np.ascontiguousarray(xT32.astype(bf16))
    gwT = np.ascontiguousarray(gate_w.T.astype(np.float32))
    in_maps = []
    for c in range(n_cores):
        e = c % N_EXPERTS
        onehot = np.zeros((128, N_EXPERTS), np.float32)
        onehot[:, e] = 1.0
        in_maps.append({
            "xT32": xT32,
            "xTbf": xTbf,
            "gwT": gwT,
            "w1T": np.ascontiguousarray(w1[e].T.astype(bf16)),
            "w2T": np.ascontiguousarray(w2[e].T.astype(bf16)),
            "b1r": np.ascontiguousarray(
                b1[e].reshape(f // 128, 128).T.astype(np.float32)),
            "b2big": np.ascontiguousarray(
                np.broadcast_to(b2[e].astype(np.float32), (128, d))),
            "esel": onehot,
        })
    return in_maps


CAP = 1152


def prep_core_inputs_sparse(x, gate_w, w1, b1, w2, b2, n_cores=N_CORES):
    bf16 = ml_dtypes.bfloat16
    n = x.shape[0] * x.shape[1]
    d = x.shape[2]
    f = w1.shape[1]
    xf = np.ascontiguousarray(x.reshape(n, d).astype(np.float32))
    xT32 = np.ascontiguousarray(xf.T)
    xThi = xT32.astype(bf16)
    xTlo = (xT32 - xThi.astype(np.float32)).astype(bf16)
    gwT = np.ascontiguousarray(gate_w.T.astype(np.float32))
    gwThi = gwT.astype(bf16)
    gwTlo = (gwT - gwThi.astype(np.float32)).astype(bf16)
    xrow = np.ascontiguousarray(xf.astype(bf16))
    # wrapped token ids: token t at [t % 16, t // 16]
    ids_in = np.ascontiguousarray(
        np.arange(n, dtype=np.float32).reshape(n // 16, 16).T)
    in_maps = []
    for c in range(n_cores):
        e = c % N_EXPERTS
        onehot = np.zeros((128, N_EXPERTS), np.float32)
        onehot[:, e] = 1.0
        in_maps.append({
            "xThi": xThi,
            "xTlo": xTlo,
            "xrow": xrow,
            "gwThi": gwThi,
            "gwTlo": gwTlo,
            "ids_in": ids_in,
            "w1T": np.ascontiguousarray(w1[e].T.astype(bf16)),
            "w2T": np.ascontiguousarray(w2[e].T.astype(bf16)),
            "b1r": np.ascontiguousarray(
                b1[e].reshape(f // 128, 128).T.astype(np.float32)),
            "b2big": np.ascontiguousarray(
                np.broadcast_to(b2[e].astype(np.float32), (128, d))),
            "esel": onehot,
        })
    return in_maps


def _combine_sparse(res, B, S, d, n):
    out = np.zeros((n, d), np.float32)
    overflow = False
    for c in range(N_CORES):
        r = res.results[c]
        cnt = int(r["cnt_out"][0, 0])
        if cnt > CAP:
            overflow = True
        ids = r["ids_out"].T.ravel()
        np.add.at(out, ids, r["yg"])
    gp_raw = res.results[0]["gp_raw"]
    TC = n // 128
    gp = (gp_raw.reshape(128, TC, N_EXPERTS)
          .transpose(1, 0, 2).reshape(B, S, N_EXPERTS))
    return out.reshape(B, S, d), gp, overflow


USE_SPARSE = True


def kernel(x, gate_w, w1, b1, w2, b2, trace=False):
    B, S, d = x.shape
    n = B * S
    if USE_SPARSE:
        key = ("sparse", n, d, D_FF, N_EXPERTS, CAP)
        if key not in _NC_CACHE:
            _NC_CACHE[key] = build_nc_sparse(n, d, D_FF, N_EXPERTS, CAP)
        nc = _NC_CACHE[key]
        in_maps = prep_core_inputs_sparse(x, gate_w, w1, b1, w2, b2)
        res = bass_utils.run_bass_kernel_spmd(
            nc, in_maps, core_ids=list(range(N_CORES)), trace=trace)
        out, gp, overflow = _combine_sparse(res, B, S, d, n)
        if trace:
            kernel.last_results = res
        if not overflow:
            return out, gp
        # capacity overflow (should not happen): fall through to dense
    nc = _get_nc()
    in_maps = prep_core_inputs(x, gate_w, w1, b1, w2, b2)
    res = bass_utils.run_bass_kernel_spmd(
        nc, in_maps, core_ids=list(range(N_CORES)), trace=trace)
    out = res.results[0]["y"].astype(np.float64)
    for c in range(1, N_CORES):
        out = out + res.results[c]["y"]
    out = out.astype(np.float32).reshape(B, S, d)
    gp_raw = res.results[0]["gp_raw"]
    TC = n // 128
    gp = (gp_raw.reshape(128, TC, N_EXPERTS)
          .transpose(1, 0, 2).reshape(B, S, N_EXPERTS))
    if trace:
        kernel.last_results = res
    return out, gp


# revision 39
# speedup vs baseline: 1.1601x; 1.1601x over previous
"""MoE FFN (8 experts, top-2, dense-combine reference) on 8 Trainium2 cores.

Strategy (expert-parallel, per the sharding hint):
  - Each core c owns expert c: computes y_c = wsel_c(t) * (GELU(x @ W1_c^T + b1_c) @ W2_c^T + b2_c)
    for ALL tokens, where wsel_c(t) is the renormalized top-2 gate weight of
    expert c for token t (0 if expert c not in token t's top-2).
  - The gate (fp32 logits -> softmax -> top-2 renormalize) is replicated on
    every core and computed on-device in fp32 (top-2 selection is
    numerically delicate: min top2/top3 prob gap is ~3e-5).
  - Host combine: out = sum_c y_c  (the expert-parallel "reduce"), gate_probs
    comes from core 0.

Layouts (host pre-transposes so the contraction dim lands on partitions):
  xT32 [D, N] fp32, xTbf [D, N] bf16, gwT [D, E] fp32, w1T [D, F] bf16,
  w2T [F, D] bf16, b1r [128, F/128] fp32 (b1r[p, fc] = b1[fc*128+p]),
  b2big [128, D] fp32 (b2 row broadcast), esel [128, E] one-hot of expert c.

Device outputs: y [N, D] fp32; gp_raw [128, (N/128)*E] fp32 packed probs
  (gp_raw[p, t128*E + e] = prob(token t128*128 + p, e)).
"""

import numpy as np
import ml_dtypes

import concourse.bacc as bacc
import concourse.mybir as mybir
import concourse.tile as tile
from concourse import bass_utils

F32 = mybir.dt.float32
BF16 = mybir.dt.bfloat16
AF = mybir.ActivationFunctionType
ALU = mybir.AluOpType
AX = mybir.AxisListType

D_MODEL, D_FF, N_EXPERTS, TOP_K = 1024, 4096, 8, 2
N_TOKENS = 4096
N_CORES = 8
T_BLK = 512


def build_nc(n_tok=N_TOKENS, d=D_MODEL, f=D_FF, e=N_EXPERTS, t_blk=T_BLK,
             act=AF.Gelu):
    KC = d // 128        # contraction chunks over d
    TC = n_tok // 128    # token 128-chunks
    FC = f // 128        # hidden-dim 128-chunks
    NTB = n_tok // t_blk # token blocks
    TCB = t_blk // 128   # 128-token chunks per block
    FG = min(512, f)     # w1 streaming group width along f
    NFG = f // FG
    FS = FG // 128

    nc = bacc.Bacc("TRN2", target_bir_lowering=False, debug=False,
                   num_devices=N_CORES)

    xT32 = nc.dram_tensor("xT32", [d, n_tok], F32, kind="ExternalInput")
    xTbf = nc.dram_tensor("xTbf", [d, n_tok], BF16, kind="ExternalInput")
    gwT = nc.dram_tensor("gwT", [d, e], F32, kind="ExternalInput")
    w1T = nc.dram_tensor("w1T", [d, f], BF16, kind="ExternalInput")
    w2T = nc.dram_tensor("w2T", [f, d], BF16, kind="ExternalInput")
    b1r = nc.dram_tensor("b1r", [128, FC], F32, kind="ExternalInput")
    b2big = nc.dram_tensor("b2big", [128, d], F32, kind="ExternalInput")
    esel = nc.dram_tensor("esel", [128, e], F32, kind="ExternalInput")
    y = nc.dram_tensor("y", [n_tok, d], F32, kind="ExternalOutput")
    gpr = nc.dram_tensor("gp_raw", [128, TC * e], F32, kind="ExternalOutput")

    xTbf_r = xTbf.ap().rearrange("(kc p) t -> p kc t", p=128)
    w1T_r = w1T.ap().rearrange("(kc p) ff -> p kc ff", p=128)
    w2T_r = w2T.ap().rearrange("(fc p) dd -> p fc dd", p=128)
    gwT_r = gwT.ap().rearrange("(kc p) ee -> p kc ee", p=128)

    with tile.TileContext(nc) as tc:
        with (
            tc.tile_pool(name="persist", bufs=1) as persist,
            tc.tile_pool(name="xblk", bufs=2) as xpool,
            tc.tile_pool(name="hblk", bufs=1) as hpool,
            tc.tile_pool(name="w1s", bufs=2) as w1pool,
            tc.tile_pool(name="ps1", bufs=2, space="PSUM") as ps1pool,
            tc.tile_pool(name="ps2", bufs=2, space="PSUM") as ps2pool,
            tc.tile_pool(name="yev", bufs=3) as ypool,
        ):
            gw_sb = persist.tile([128, KC, e], F32)
            nc.sync.dma_start(gw_sb[:], gwT_r)
            b1_sb = persist.tile([128, FC], F32)
            nc.sync.dma_start(b1_sb[:], b1r.ap())
            b2_sb = persist.tile([128, d], F32)
            nc.sync.dma_start(b2_sb[:], b2big.ap())
            esel_sb = persist.tile([128, e], F32)
            nc.sync.dma_start(esel_sb[:], esel.ap())
            w2sb = persist.tile([128, FC, d], BF16)
            nc.sync.dma_start(w2sb[:], w2T_r)
            # gate state
            exp3 = persist.tile([128, TC, e], F32)
            probs3 = persist.tile([128, TC, e], F32)
            wsel = persist.tile([128, TC], F32)

            def emit_mm1(tb):
                """h^T for token block tb: htb[p, fc, t] = gelu(x @ W1^T + b1)."""
                xtb = xpool.tile([128, KC, t_blk], BF16)
                nc.sync.dma_start(
                    xtb[:], xTbf_r[:, :, tb * t_blk:(tb + 1) * t_blk])
                htb = hpool.tile([128, FC, t_blk], BF16)
                for fg in range(NFG):
                    w1g = w1pool.tile([128, KC, FG], BF16)
                    nc.sync.dma_start(
                        w1g[:], w1T_r[:, :, fg * FG:(fg + 1) * FG])
                    for fs in range(FS):
                        fc = fg * FS + fs
                        ps = ps1pool.tile([128, t_blk], F32)
                        for kc in range(KC):
                            nc.tensor.matmul(
                                ps[:],
                                w1g[:, kc, fs * 128:(fs + 1) * 128],
                                xtb[:, kc, :],
                                start=(kc == 0), stop=(kc == KC - 1))
                        nc.scalar.activation(
                            htb[:, fc, :], ps[:], act,
                            bias=b1_sb[:, fc:fc + 1], scale=1.0)
                return htb

            def emit_gate():
                with (
                    tc.tile_pool(name="gx", bufs=4) as gxp,
                    tc.tile_pool(name="gps", bufs=2, space="PSUM") as gpp,
                    tc.tile_pool(name="gtmp", bufs=1) as gtp,
                ):
                    for t in range(TC):
                        ps = gpp.tile([128, e], F32)
                        for kc in range(KC):
                            xt = gxp.tile([128, 128], F32)
                            nc.sync.dma_start(
                                xt[:],
                                xT32.ap()[kc * 128:(kc + 1) * 128,
                                          t * 128:(t + 1) * 128])
                            nc.tensor.matmul(
                                ps[:], xt[:], gw_sb[:, kc, :],
                                start=(kc == 0), stop=(kc == KC - 1))
                        nc.vector.tensor_copy(exp3[:, t, :], ps[:])
                    # softmax over e (innermost)
                    mx = gtp.tile([128, TC], F32)
                    nc.vector.tensor_reduce(mx[:], exp3[:], AX.X, ALU.max)
                    nc.vector.tensor_tensor(
                        exp3[:], exp3[:], mx[:].to_broadcast([128, TC, e]),
                        ALU.subtract)
                    nc.scalar.activation(exp3[:], exp3[:], AF.Exp)
                    s = gtp.tile([128, TC], F32)
                    nc.vector.tensor_reduce(s[:], exp3[:], AX.X, ALU.add)
                    rs = gtp.tile([128, TC], F32)
                    nc.vector.reciprocal(rs[:], s[:])
                    nc.vector.tensor_tensor(
                        probs3[:], exp3[:], rs[:].to_broadcast([128, TC, e]),
                        ALU.mult)
                    nc.sync.dma_start(gpr.ap(), probs3[:])
                    # top-2 over e: m1e = max(exp), zap, m2e = 2nd max
                    m1e = gtp.tile([128, TC], F32)
                    nc.vector.tensor_reduce(m1e[:], exp3[:], AX.X, ALU.max)
                    eq1 = gtp.tile([128, TC, e], F32)
                    nc.vector.tensor_tensor(
                        eq1[:], exp3[:], m1e[:].to_broadcast([128, TC, e]),
                        ALU.is_ge)
                    t1 = gtp.tile([128, TC, e], F32)
                    nc.vector.tensor_tensor(t1[:], eq1[:], exp3[:], ALU.mult)
                    nc.vector.tensor_tensor(t1[:], exp3[:], t1[:],
                                            ALU.subtract)
                    m2e = gtp.tile([128, TC], F32)
                    nc.vector.tensor_reduce(m2e[:], t1[:], AX.X, ALU.max)
                    # top-2 mask, this core's expert, renormalize:
                    # wsel = exp * (exp >= m2e) . esel / (1 + m2e)
                    ge2 = gtp.tile([128, TC, e], F32)
                    nc.vector.tensor_tensor(
                        ge2[:], exp3[:], m2e[:].to_broadcast([128, TC, e]),
                        ALU.is_ge)
                    nc.vector.tensor_tensor(ge2[:], ge2[:], exp3[:], ALU.mult)
                    nc.vector.tensor_tensor(
                        ge2[:], ge2[:],
                        esel_sb[:, None, :].to_broadcast([128, TC, e]),
                        ALU.mult)
                    nc.vector.tensor_reduce(wsel[:], ge2[:], AX.X, ALU.add)
                    m2p1 = gtp.tile([128, TC], F32)
                    nc.vector.tensor_scalar_add(m2p1[:], m2e[:], 1.0)
                    winv = gtp.tile([128, TC], F32)
                    nc.vector.reciprocal(winv[:], m2p1[:])
                    nc.vector.tensor_tensor(wsel[:], wsel[:], winv[:],
                                            ALU.mult)

            def emit_mm2(tb, htb):
                """y rows for block tb: y = ((h @ W2^T) + b2) * wsel."""
                for tcb in range(TCB):
                    tg = tb * TCB + tcb
                    ps2 = ps2pool.tile([128, d], F32)
                    dw = min(512, d)
                    for fc in range(FC):
                        lhsT = htb[:, fc, tcb * 128:(tcb + 1) * 128]
                        for dh in range(d // dw):
                            nc.tensor.matmul(
                                ps2[:, dh * dw:(dh + 1) * dw],
                                lhsT,
                                w2sb[:, fc, dh * dw:(dh + 1) * dw],
                                start=(fc == 0), stop=(fc == FC - 1))
                    yb = ypool.tile([128, d], F32)
                    nc.vector.tensor_add(yb[:], ps2[:], b2_sb[:])
                    nc.vector.tensor_scalar_mul(yb[:], yb[:],
                                                wsel[:, tg:tg + 1])
                    nc.sync.dma_start(
                        y.ap()[tg * 128:(tg + 1) * 128, :], yb[:])

            htb0 = emit_mm1(0)
            emit_gate()
            emit_mm2(0, htb0)
            for tb in range(1, NTB):
                htb = emit_mm1(tb)
                emit_mm2(tb, htb)

    nc.compile()
    return nc


def build_nc_sparse(n_tok=N_TOKENS, d=D_MODEL, f=D_FF, e=N_EXPERTS,
                    cap=1152, act=AF.Gelu, stage=5):
    """Sparse expert-parallel: each core gathers only the tokens routed to
    its expert (top-2 of the gate), computes the FFN on those, and writes
    compacted rows + their token ids; host scatter-adds."""
    import concourse.bass_isa as bass_isa  # noqa: F401
    I32 = mybir.dt.int32
    I16 = mybir.dt.int16
    U32 = mybir.dt.uint32

    KC = d // 128
    TC = n_tok // 128
    FC = f // 128
    NW = n_tok // 16        # wrapped free size for sparse_gather input
    CW = cap // 16          # wrapped free size for compacted lists
    CPC = cap // 128        # 128-token chunks of capacity
    NTG = n_tok // 512      # gate t-groups

    assert cap % 128 == 0 and CW <= 512

    nc = bacc.Bacc("TRN2", target_bir_lowering=False, debug=False,
                   num_devices=N_CORES)

    xThi = nc.dram_tensor("xThi", [d, n_tok], BF16, kind="ExternalInput")
    xTlo = nc.dram_tensor("xTlo", [d, n_tok], BF16, kind="ExternalInput")
    xrow = nc.dram_tensor("xrow", [n_tok, d], BF16, kind="ExternalInput")
    gwThi = nc.dram_tensor("gwThi", [d, e], BF16, kind="ExternalInput")
    gwTlo = nc.dram_tensor("gwTlo", [d, e], BF16, kind="ExternalInput")
    w1T = nc.dram_tensor("w1T", [d, f], BF16, kind="ExternalInput")
    w2T = nc.dram_tensor("w2T", [f, d], BF16, kind="ExternalInput")
    b1r = nc.dram_tensor("b1r", [128, FC], F32, kind="ExternalInput")
    b2big = nc.dram_tensor("b2big", [128, d], F32, kind="ExternalInput")
    esel = nc.dram_tensor("esel", [128, e], F32, kind="ExternalInput")
    ids_in = nc.dram_tensor("ids_in", [16, NW], F32, kind="ExternalInput")
    yg = nc.dram_tensor("yg", [cap, d], F32, kind="ExternalOutput")
    ids_out = nc.dram_tensor("ids_out", [16, CW], I32, kind="ExternalOutput")
    cnt_out = nc.dram_tensor("cnt_out", [1, 1], U32, kind="ExternalOutput")
    gpr = nc.dram_tensor("gp_raw", [128, TC * e], F32, kind="ExternalOutput")

    xThi_r = xThi.ap().rearrange("(kc p) t -> p kc t", p=128)
    xTlo_r = xTlo.ap().rearrange("(kc p) t -> p kc t", p=128)
    w1T_r = w1T.ap().rearrange("(kc p) ff -> p kc ff", p=128)
    w2T_r = w2T.ap().rearrange("(fc p) dd -> p fc dd", p=128)
    gwThi_r = gwThi.ap().rearrange("(kc p) ee -> p kc ee", p=128)
    gwTlo_r = gwTlo.ap().rearrange("(kc p) ee -> p kc ee", p=128)

    from concourse import library_config

    def _emit(tc):
        with (
            tc.tile_pool(name="persist", bufs=1) as persist,
            tc.tile_pool(name="route", bufs=1) as route,
        ):
            gwhi_sb = persist.tile([128, KC, e], BF16)
            nc.sync.dma_start(gwhi_sb[:], gwThi_r)
            gwlo_sb = persist.tile([128, KC, e], BF16)
            nc.sync.dma_start(gwlo_sb[:], gwTlo_r)
            b1_sb = persist.tile([128, FC], F32)
            nc.sync.dma_start(b1_sb[:], b1r.ap())
            b2_sb = persist.tile([128, d], F32)
            nc.sync.dma_start(b2_sb[:], b2big.ap())
            esel_sb = persist.tile([128, e], F32)
            nc.sync.dma_start(esel_sb[:], esel.ap())
            w2sb = persist.tile([128, FC, d], BF16)
            exp3 = persist.tile([128, TC, e], F32)
            probs3 = persist.tile([128, TC, e], F32)
            wsel = persist.tile([128, TC], F32)

            # ---------------- gate ----------------
            # processed per 512-token group; the softmax/top-2 chain for
            # group g runs on DVE/ACT while the PE computes group g+1
            with (
                tc.tile_pool(name="gx", bufs=3) as gxp,
                tc.tile_pool(name="gps", bufs=4, space="PSUM") as gpp,
                tc.tile_pool(name="gtmp", bufs=2) as gtp,
            ):
                for tg in range(NTG):
                    # split-bf16 gate: logits = xhi@ghi + xlo@ghi + xhi@glo
                    # (~2^-16 relative accuracy; dropped xlo@glo is ~2^-24)
                    gxh = gxp.tile([128, KC, 512], BF16, tag="gxh")
                    nc.sync.dma_start(
                        gxh[:], xThi_r[:, :, tg * 512:(tg + 1) * 512])
                    gxl = gxp.tile([128, KC, 512], BF16, tag="gxl")
                    nc.sync.dma_start(
                        gxl[:], xTlo_r[:, :, tg * 512:(tg + 1) * 512])
                    for ts4 in range(4):
                        t = tg * 4 + ts4
                        ps = gpp.tile([128, e], F32)
                        for kc in range(KC):
                            xh = gxh[:, kc, ts4 * 128:(ts4 + 1) * 128]
                            xl = gxl[:, kc, ts4 * 128:(ts4 + 1) * 128]
                            nc.tensor.matmul(
                                ps[:], xh, gwhi_sb[:, kc, :],
                                start=(kc == 0), stop=False)
                            nc.tensor.matmul(
                                ps[:], xh, gwlo_sb[:, kc, :],
                                start=False, stop=False)
                            nc.tensor.matmul(
                                ps[:], xl, gwhi_sb[:, kc, :],
                                start=False, stop=(kc == KC - 1))
                        nc.vector.tensor_copy(exp3[:, t, :], ps[:])
                    # softmax + top-2 for this group's token slice
                    TG4 = 4
                    sl = slice(tg * TG4, (tg + 1) * TG4)
                    E3 = exp3[:, sl, :]
                    P3 = probs3[:, sl, :]
                    mx = gtp.tile([128, TG4], F32, tag="mx")
                    nc.vector.tensor_reduce(mx[:], E3, AX.X, ALU.max)
                    nc.vector.tensor_tensor(
                        E3, E3, mx[:].to_broadcast([128, TG4, e]),
                        ALU.subtract)
                    nc.scalar.activation(E3, E3, AF.Exp)
                    s = gtp.tile([128, TG4], F32, tag="s")
                    nc.vector.tensor_reduce(s[:], E3, AX.X, ALU.add)
                    rs = gtp.tile([128, TG4], F32, tag="rs")
                    nc.vector.reciprocal(rs[:], s[:])
                    nc.vector.tensor_tensor(
                        P3, E3, rs[:].to_broadcast([128, TG4, e]), ALU.mult)
                    m1e = gtp.tile([128, TG4], F32, tag="m1e")
                    nc.vector.tensor_reduce(m1e[:], E3, AX.X, ALU.max)
                    eq1 = gtp.tile([128, TG4, e], F32, tag="eq1")
                    nc.vector.tensor_tensor(
                        eq1[:], E3, m1e[:].to_broadcast([128, TG4, e]),
                        ALU.is_ge)
                    t1 = gtp.tile([128, TG4, e], F32, tag="t1")
                    nc.vector.tensor_tensor(t1[:], eq1[:], E3, ALU.mult)
                    nc.vector.tensor_tensor(t1[:], E3, t1[:], ALU.subtract)
                    m2e = gtp.tile([128, TG4], F32, tag="m2e")
                    nc.vector.tensor_reduce(m2e[:], t1[:], AX.X, ALU.max)
                    ge2 = gtp.tile([128, TG4, e], F32, tag="ge2")
                    nc.vector.tensor_tensor(
                        ge2[:], E3, m2e[:].to_broadcast([128, TG4, e]),
                        ALU.is_ge)
                    nc.vector.tensor_tensor(ge2[:], ge2[:], E3, ALU.mult)
                    nc.vector.tensor_tensor(
                        ge2[:], ge2[:],
                        esel_sb[:, None, :].to_broadcast([128, TG4, e]),
                        ALU.mult)
                    nc.vector.tensor_reduce(wsel[:, sl], ge2[:], AX.X,
                                            ALU.add)
                    m2p1 = gtp.tile([128, TG4], F32, tag="m2p1")
                    nc.vector.tensor_scalar_add(m2p1[:], m2e[:], 1.0)
                    winv = gtp.tile([128, TG4], F32, tag="winv")
                    nc.vector.reciprocal(winv[:], m2p1[:])
                    nc.vector.tensor_tensor(wsel[:, sl], wsel[:, sl],
                                            winv[:], ALU.mult)
                nc.sync.dma_start(gpr.ap(), probs3[:])
            if stage <= 1:
                return

            # ---------------- routing: compact this expert's tokens --------
            # wrapped layout: token t at [t % 16, t // 16]
            wselw = route.tile([16, NW], F32)
            wselw3 = wselw[:].rearrange("q (ft a) -> q ft a", a=8)
            for a in range(8):
                # partition-base-16a access: engines need quadrant-aligned
                # partition starts, so shuffle via DMA
                nc.sync.dma_start(
                    wselw3[:, :, a], wsel[16 * a:16 * (a + 1), :])
            # wrapped token ids [16, NW] as f32, provided by the host (a
            # static iota; avoids a GpSimd library switch for InstIota)
            ids_f = route.tile([16, NW], F32)
            nc.sync.dma_start(ids_f[:], ids_in.ap())
            maskw = route.tile([16, NW], F32)
            nc.vector.tensor_scalar(maskw[:], wselw[:], 0.0, None,
                                    op0=ALU.is_gt)
            # sel_id = (id+1)*mask - 1  (id where selected, -1 elsewhere)
            sel_id = route.tile([16, NW], F32)
            nc.vector.tensor_scalar(sel_id[:], ids_f[:], 1.0, None,
                                    op0=ALU.add)
            nc.vector.tensor_tensor(sel_id[:], sel_id[:], maskw[:], ALU.mult)
            nc.vector.tensor_scalar(sel_id[:], sel_id[:], 1.0, None,
                                    op0=ALU.subtract)
            # sel_w = w + (mask-1)  (w>0 where selected, -1 elsewhere)
            sel_w = route.tile([16, NW], F32)
            m1t = route.tile([16, NW], F32)
            nc.vector.tensor_scalar(m1t[:], maskw[:], 1.0, None,
                                    op0=ALU.subtract)
            nc.vector.tensor_add(sel_w[:], wselw[:], m1t[:])

            ids_c = route.tile([16, CW], F32)
            cnt = route.tile([1, 1], U32)
            nc.vector.memset(ids_c[:], -1.0)
            nc.gpsimd.sparse_gather(ids_c[:], sel_id[:], num_found=cnt[:])
            w_c = route.tile([16, CW], F32)
            cnt2 = route.tile([1, 1], U32)
            nc.vector.memset(w_c[:], -1.0)
            nc.gpsimd.sparse_gather(w_c[:], sel_w[:], num_found=cnt2[:])
            nc.sync.dma_start(cnt_out.ap(), cnt[:])
            # On HW the tail beyond num_found is uninitialized garbage (the
            # sim fills -1): force tail slots to 0 by position, overwriting
            # whatever junk is there (ids -> token 0, weights -> 0).
            # cnt is broadcast to 16 partitions with a K=1 ones matmul on the
            # (idle) PE instead of gpsimd.partition_broadcast — keeps the
            # GpSimd op sequence inside a single ucode library.
            cnt_f = route.tile([1, 1], F32)
            nc.vector.tensor_copy(cnt_f[:], cnt[:])
            ones16 = route.tile([1, 16], F32)
            nc.vector.memset(ones16[:], 1.0)
            with tc.tile_pool(name="cps", bufs=1, space="PSUM") as cpsp:
                cps = cpsp.tile([16, 1], F32)
                nc.tensor.matmul(cps[:], ones16[:], cnt_f[:],
                                 start=True, stop=True)
                cntb = route.tile([16, 1], F32)
                nc.vector.tensor_copy(cntb[:], cps[:])
            notkeep = route.tile([16, CW], U32)
            nc.vector.tensor_tensor(notkeep[:], ids_f[:, :CW],
                                    cntb[:].to_broadcast([16, CW]), ALU.is_ge)
            zeros16 = route.tile([16, CW], F32)
            nc.vector.memset(zeros16[:], 0.0)
            nc.vector.copy_predicated(ids_c[:], notkeep[:], zeros16[:])
            nc.vector.copy_predicated(w_c[:], notkeep[:], zeros16[:])
            ids32 = route.tile([16, CW], I32)
            nc.vector.tensor_copy(ids32[:], ids_c[:])
            nc.sync.dma_start(ids_out.ap(), ids32[:])
            ids16 = route.tile([16, CW], I16)
            nc.vector.tensor_copy(ids16[:], ids_c[:])
            idx128 = route.tile([128, CW], I16)
            for a in range(8):
                nc.sync.dma_start(idx128[16 * a:16 * (a + 1), :], ids16[:])
            # per-128-chunk gate weights: wpart[16a+q, c] = w_c[q, 8c+a]
            wpart = route.tile([128, CPC], F32)
            w_c3 = w_c[:].rearrange("q (c a) -> q c a", a=8)
            for a in range(8):
                nc.sync.dma_start(wpart[16 * a:16 * (a + 1), :],
                                  w_c3[:, :, a])

            if stage <= 2:
                return

            # ---------------- gather x rows ----------------
            with (
                tc.tile_pool(name="xg", bufs=1) as xgp,
                tc.tile_pool(name="hg", bufs=1) as hgp,
                tc.tile_pool(name="w1s", bufs=2) as w1pool,
                tc.tile_pool(name="ps1", bufs=2, space="PSUM") as ps1pool,
                tc.tile_pool(name="ps2", bufs=2, space="PSUM") as ps2pool,
                tc.tile_pool(name="yev", bufs=3) as ypool,
            ):
                # chunks of the capacity: <=512 wide for the matmul free-dim
                # limit AND for dma_gather (one gather's s2m descriptor
                # count must fit the 128-entry SWDGE ring -> <=512 idxs)
                # smallest chunk first: its descriptor generation + transfer
                # finish quickest, so mm1's first matmuls start sooner
                widths = [256, 512, 512] if cap == 1280 else [256, 512, 384]
                if cap not in (1280, 1152):
                    widths = []
                    off = 0
                    while off < cap:
                        widths.append(min(512, cap - off))
                        off += widths[-1]
                chunks = []
                off = 0
                for cwid in widths:
                    chunks.append((off, cwid))
                    off += cwid
                assert off == cap

                xg_tiles = {}
                for (off, cwid) in chunks:
                    xgc = xgp.tile([128, KC, cwid], BF16, tag=f"xg{off}")
                    nc.gpsimd.dma_gather(
                        xgc[:], xrow.ap(),
                        idx128[:, off // 16:(off + cwid) // 16],
                        num_idxs=cwid, num_idxs_reg=cwid, elem_size=d,
                        transpose=True)
                    xg_tiles[off] = xgc
                # w2 (8MB) is first needed by mm2. The GpSimd ucode-library
                # switch before the gathers drains ALL in-flight SWDGE DMA,
                # so an early-running w2 transfer would stall it ~22us.
                # Write a dummy sliver of w2sb from the last gather's output
                # first: the WAW dependency forces the w2 DMA after the
                # gathers have issued.
                last_xg = xg_tiles[chunks[-1][0]]
                nc.vector.tensor_copy(w2sb[:, 0, 0:2], last_xg[:, 0, 0:2])
                nc.sync.dma_start(w2sb[:], w2T_r)
                if stage <= 3:
                    ytmp = ypool.tile([128, d], F32)
                    nc.vector.tensor_copy(ytmp[:, 0:cap // 4],
                                          xg_tiles[0][:, 0, 0:cap // 4])
                    nc.sync.dma_start(yg.ap()[0:128, :], ytmp[:])
                    return

                hg = hgp.tile([128, FC, cap], BF16)
                FG = min(512, f)
                for fg in range(f // FG):
                    w1g = w1pool.tile([128, KC, FG], BF16)
                    nc.sync.dma_start(
                        w1g[:], w1T_r[:, :, fg * FG:(fg + 1) * FG])
                    for fs in range(FG // 128):
                        fc = fg * (FG // 128) + fs
                        for (off, cwid) in chunks:
                            ps = ps1pool.tile([128, 512], F32)
                            for kc in range(KC):
                                nc.tensor.matmul(
                                    ps[:, :cwid],
                                    w1g[:, kc, fs * 128:(fs + 1) * 128],
                                    xg_tiles[off][:, kc, :],
                                    start=(kc == 0), stop=(kc == KC - 1))
                            nc.scalar.activation(
                                hg[:, fc, off:off + cwid], ps[:, :cwid],
                                act, bias=b1_sb[:, fc:fc + 1], scale=1.0)

                if stage <= 4:
                    ytmp = ypool.tile([128, d], F32)
                    nc.vector.tensor_copy(ytmp[:, 0:cap // 4],
                                          hg[:, 0, 0:cap // 4])
                    nc.sync.dma_start(yg.ap()[0:128, :], ytmp[:])
                    return

                dw = min(512, d)
                for tcb in range(CPC):
                    ps2 = ps2pool.tile([128, d], F32)
                    for fc in range(FC):
                        lhsT = hg[:, fc, tcb * 128:(tcb + 1) * 128]
                        for dh in range(d // dw):
                            nc.tensor.matmul(
                                ps2[:, dh * dw:(dh + 1) * dw],
                                lhsT,
                                w2sb[:, fc, dh * dw:(dh + 1) * dw],
                                start=(fc == 0), stop=(fc == FC - 1))
                    yb = ypool.tile([128, d], F32)
                    nc.vector.tensor_add(yb[:], ps2[:], b2_sb[:])
                    nc.vector.tensor_scalar_mul(yb[:], yb[:],
                                                wpart[:, tcb:tcb + 1])
                    nc.sync.dma_start(
                        yg.ap()[tcb * 128:(tcb + 1) * 128, :], yb[:])

    with tile.TileContext(nc) as tc:
        _emit(tc)
    nc.compile()
    return nc


_NC_CACHE = {}


def _get_nc():
    key = (N_TOKENS, D_MODEL, D_FF, N_EXPERTS, T_BLK)
    if key not in _NC_CACHE:
        _NC_CACHE[key] = build_nc(*key)
    return _NC_CACHE[key]


def prep_core_inputs(x, gate_w, w1, b1, w2, b2, n_cores=N_CORES):
    """Host-side sharding: per-core input dicts (expert-parallel)."""
    bf16 = ml_dtypes.bfloat16
    n = x.shape[0] * x.shape[1]
    d = x.shape[2]
    f = w1.shape[1]
    xT32 = np.ascontiguousarray(x.reshape(n, d).T.astype(np.float32))
    xTbf = np.ascontiguousarray(xT32.astype(bf16))
    gwT = np.ascontiguousarray(gate_w.T.astype(np.float32))
    in_maps = []
    for c in range(n_cores):
        e = c % N_EXPERTS
        onehot = np.zeros((128, N_EXPERTS), np.float32)
        onehot[:, e] = 1.0
        in_maps.append({
            "xT32": xT32,
            "xTbf": xTbf,
            "gwT": gwT,
            "w1T": np.ascontiguousarray(w1[e].T.astype(bf16)),
            "w2T": np.ascontiguousarray(w2[e].T.astype(bf16)),
            "b1r": np.ascontiguousarray(
                b1[e].reshape(f // 128, 128).T.astype(np.float32)),
            "b2big": np.ascontiguousarray(
                np.broadcast_to(b2[e].astype(np.float32), (128, d))),
            "esel": onehot,
        })
    return in_maps


CAP = 1152


def prep_core_inputs_sparse(x, gate_w, w1, b1, w2, b2, n_cores=N_CORES):
    bf16 = ml_dtypes.bfloat16
    n = x.shape[0] * x.shape[1]
    d = x.shape[2]
    f = w1.shape[1]
    xf = np.ascontiguousarray(x.reshape(n, d).astype(np.float32))
    xT32 = np.ascontiguousarray(xf.T)
    xThi = xT32.astype(bf16)
    xTlo = (xT32 - xThi.astype(np.float32)).astype(bf16)
    gwT = np.ascontiguousarray(gate_w.T.astype(np.float32))
    gwThi = gwT.astype(bf16)
    gwTlo = (gwT - gwThi.astype(np.float32)).astype(bf16)
    xrow = np.ascontiguousarray(xf.astype(bf16))
    # wrapped token ids: token t at [t % 16, t // 16]
    ids_in = np.ascontiguousarray(
        np.arange(n, dtype=np.float32).reshape(n // 16, 16).T)
    in_maps = []
    for c in range(n_cores):
        e = c % N_EXPERTS
        onehot = np.zeros((128, N_EXPERTS), np.float32)
        onehot[:, e] = 1.0
        in_maps.append({
            "xThi": xThi,
            "xTlo": xTlo,
            "xrow": xrow,
            "gwThi": gwThi,
            "gwTlo": gwTlo,
            "ids_in": ids_in,
            "w1T": np.ascontiguousarray(w1[e].T.astype(bf16)),
            "w2T": np.ascontiguousarray(w2[e].T.astype(bf16)),
            "b1r": np.ascontiguousarray(
                b1[e].reshape(f // 128, 128).T.astype(np.float32)),
            "b2big": np.ascontiguousarray(
                np.broadcast_to(b2[e].astype(np.float32), (128, d))),
            "esel": onehot,
        })
    return in_maps


def _combine_sparse(res, B, S, d, n):
    out = np.zeros((n, d), np.float32)
    overflow = False
    for c in range(N_CORES):
        r = res.results[c]
        cnt = int(r["cnt_out"][0, 0])
        if cnt > CAP:
            overflow = True
        ids = r["ids_out"].T.ravel()
        np.add.at(out, ids, r["yg"])
    gp_raw = res.results[0]["gp_raw"]
    TC = n // 128
    gp = (gp_raw.reshape(128, TC, N_EXPERTS)
          .transpose(1, 0, 2).reshape(B, S, N_EXPERTS))
    return out.reshape(B, S, d), gp, overflow


USE_SPARSE = True


def kernel(x, gate_w, w1, b1, w2, b2, trace=False):
    B, S, d = x.shape
    n = B * S
    if USE_SPARSE:
        key = ("sparse", n, d, D_FF, N_EXPERTS, CAP)
        if key not in _NC_CACHE:
            _NC_CACHE[key] = build_nc_sparse(n, d, D_FF, N_EXPERTS, CAP)
        nc = _NC_CACHE[key]
        in_maps = prep_core_inputs_sparse(x, gate_w, w1, b1, w2, b2)
        res = bass_utils.run_bass_kernel_spmd(
            nc, in_maps, core_ids=list(range(N_CORES)), trace=trace)
        out, gp, overflow = _combine_sparse(res, B, S, d, n)
        if trace:
            kernel.last_results = res
        if not overflow:
            return out, gp
        # capacity overflow (should not happen): fall through to dense
    nc = _get_nc()
    in_maps = prep_core_inputs(x, gate_w, w1, b1, w2, b2)
    res = bass_utils.run_bass_kernel_spmd(
        nc, in_maps, core_ids=list(range(N_CORES)), trace=trace)
    out = res.results[0]["y"].astype(np.float64)
    for c in range(1, N_CORES):
        out = out + res.results[c]["y"]
    out = out.astype(np.float32).reshape(B, S, d)
    gp_raw = res.results[0]["gp_raw"]
    TC = n // 128
    gp = (gp_raw.reshape(128, TC, N_EXPERTS)
          .transpose(1, 0, 2).reshape(B, S, N_EXPERTS))
    if trace:
        kernel.last_results = res
    return out, gp


# revision 40
# speedup vs baseline: 1.2739x; 1.0981x over previous
"""MoE FFN (8 experts, top-2, dense-combine reference) on 8 Trainium2 cores.

Strategy (expert-parallel, per the sharding hint):
  - Each core c owns expert c: computes y_c = wsel_c(t) * (GELU(x @ W1_c^T + b1_c) @ W2_c^T + b2_c)
    for ALL tokens, where wsel_c(t) is the renormalized top-2 gate weight of
    expert c for token t (0 if expert c not in token t's top-2).
  - The gate (fp32 logits -> softmax -> top-2 renormalize) is replicated on
    every core and computed on-device in fp32 (top-2 selection is
    numerically delicate: min top2/top3 prob gap is ~3e-5).
  - Host combine: out = sum_c y_c  (the expert-parallel "reduce"), gate_probs
    comes from core 0.

Layouts (host pre-transposes so the contraction dim lands on partitions):
  xT32 [D, N] fp32, xTbf [D, N] bf16, gwT [D, E] fp32, w1T [D, F] bf16,
  w2T [F, D] bf16, b1r [128, F/128] fp32 (b1r[p, fc] = b1[fc*128+p]),
  b2big [128, D] fp32 (b2 row broadcast), esel [128, E] one-hot of expert c.

Device outputs: y [N, D] fp32; gp_raw [128, (N/128)*E] fp32 packed probs
  (gp_raw[p, t128*E + e] = prob(token t128*128 + p, e)).
"""

import numpy as np
import ml_dtypes

import concourse.bacc as bacc
import concourse.mybir as mybir
import concourse.tile as tile
from concourse import bass_utils

F32 = mybir.dt.float32
BF16 = mybir.dt.bfloat16
AF = mybir.ActivationFunctionType
ALU = mybir.AluOpType
AX = mybir.AxisListType

D_MODEL, D_FF, N_EXPERTS, TOP_K = 1024, 4096, 8, 2
N_TOKENS = 4096
N_CORES = 8
T_BLK = 512


def build_nc(n_tok=N_TOKENS, d=D_MODEL, f=D_FF, e=N_EXPERTS, t_blk=T_BLK,
             act=AF.Gelu):
    KC = d // 128        # contraction chunks over d
    TC = n_tok // 128    # token 128-chunks
    FC = f // 128        # hidden-dim 128-chunks
    NTB = n_tok // t_blk # token blocks
    TCB = t_blk // 128   # 128-token chunks per block
    FG = min(512, f)     # w1 streaming group width along f
    NFG = f // FG
    FS = FG // 128

    nc = bacc.Bacc("TRN2", target_bir_lowering=False, debug=False,
                   num_devices=N_CORES)

    xT32 = nc.dram_tensor("xT32", [d, n_tok], F32, kind="ExternalInput")
    xTbf = nc.dram_tensor("xTbf", [d, n_tok], BF16, kind="ExternalInput")
    gwT = nc.dram_tensor("gwT", [d, e], F32, kind="ExternalInput")
    w1T = nc.dram_tensor("w1T", [d, f], BF16, kind="ExternalInput")
    w2T = nc.dram_tensor("w2T", [f, d], BF16, kind="ExternalInput")
    b1r = nc.dram_tensor("b1r", [128, FC], F32, kind="ExternalInput")
    b2big = nc.dram_tensor("b2big", [128, d], F32, kind="ExternalInput")
    esel = nc.dram_tensor("esel", [128, e], F32, kind="ExternalInput")
    y = nc.dram_tensor("y", [n_tok, d], F32, kind="ExternalOutput")
    gpr = nc.dram_tensor("gp_raw", [128, TC * e], F32, kind="ExternalOutput")

    xTbf_r = xTbf.ap().rearrange("(kc p) t -> p kc t", p=128)
    w1T_r = w1T.ap().rearrange("(kc p) ff -> p kc ff", p=128)
    w2T_r = w2T.ap().rearrange("(fc p) dd -> p fc dd", p=128)
    gwT_r = gwT.ap().rearrange("(kc p) ee -> p kc ee", p=128)

    with tile.TileContext(nc) as tc:
        with (
            tc.tile_pool(name="persist", bufs=1) as persist,
            tc.tile_pool(name="xblk", bufs=2) as xpool,
            tc.tile_pool(name="hblk", bufs=1) as hpool,
            tc.tile_pool(name="w1s", bufs=2) as w1pool,
            tc.tile_pool(name="ps1", bufs=2, space="PSUM") as ps1pool,
            tc.tile_pool(name="ps2", bufs=2, space="PSUM") as ps2pool,
            tc.tile_pool(name="yev", bufs=3) as ypool,
        ):
            gw_sb = persist.tile([128, KC, e], F32)
            nc.sync.dma_start(gw_sb[:], gwT_r)
            b1_sb = persist.tile([128, FC], F32)
            nc.sync.dma_start(b1_sb[:], b1r.ap())
            b2_sb = persist.tile([128, d], F32)
            nc.sync.dma_start(b2_sb[:], b2big.ap())
            esel_sb = persist.tile([128, e], F32)
            nc.sync.dma_start(esel_sb[:], esel.ap())
            w2sb = persist.tile([128, FC, d], BF16)
            nc.sync.dma_start(w2sb[:], w2T_r)
            # gate state
            exp3 = persist.tile([128, TC, e], F32)
            probs3 = persist.tile([128, TC, e], F32)
            wsel = persist.tile([128, TC], F32)

            def emit_mm1(tb):
                """h^T for token block tb: htb[p, fc, t] = gelu(x @ W1^T + b1)."""
                xtb = xpool.tile([128, KC, t_blk], BF16)
                nc.sync.dma_start(
                    xtb[:], xTbf_r[:, :, tb * t_blk:(tb + 1) * t_blk])
                htb = hpool.tile([128, FC, t_blk], BF16)
                for fg in range(NFG):
                    w1g = w1pool.tile([128, KC, FG], BF16)
                    nc.sync.dma_start(
                        w1g[:], w1T_r[:, :, fg * FG:(fg + 1) * FG])
                    for fs in range(FS):
                        fc = fg * FS + fs
                        ps = ps1pool.tile([128, t_blk], F32)
                        for kc in range(KC):
                            nc.tensor.matmul(
                                ps[:],
                                w1g[:, kc, fs * 128:(fs + 1) * 128],
                                xtb[:, kc, :],
                                start=(kc == 0), stop=(kc == KC - 1))
                        nc.scalar.activation(
                            htb[:, fc, :], ps[:], act,
                            bias=b1_sb[:, fc:fc + 1], scale=1.0)
                return htb

            def emit_gate():
                with (
                    tc.tile_pool(name="gx", bufs=4) as gxp,
                    tc.tile_pool(name="gps", bufs=2, space="PSUM") as gpp,
                    tc.tile_pool(name="gtmp", bufs=1) as gtp,
                ):
                    for t in range(TC):
                        ps = gpp.tile([128, e], F32)
                        for kc in range(KC):
                            xt = gxp.tile([128, 128], F32)
                            nc.sync.dma_start(
                                xt[:],
                                xT32.ap()[kc * 128:(kc + 1) * 128,
                                          t * 128:(t + 1) * 128])
                            nc.tensor.matmul(
                                ps[:], xt[:], gw_sb[:, kc, :],
                                start=(kc == 0), stop=(kc == KC - 1))
                        nc.vector.tensor_copy(exp3[:, t, :], ps[:])
                    # softmax over e (innermost)
                    mx = gtp.tile([128, TC], F32)
                    nc.vector.tensor_reduce(mx[:], exp3[:], AX.X, ALU.max)
                    nc.vector.tensor_tensor(
                        exp3[:], exp3[:], mx[:].to_broadcast([128, TC, e]),
                        ALU.subtract)
                    nc.scalar.activation(exp3[:], exp3[:], AF.Exp)
                    s = gtp.tile([128, TC], F32)
                    nc.vector.tensor_reduce(s[:], exp3[:], AX.X, ALU.add)
                    rs = gtp.tile([128, TC], F32)
                    nc.vector.reciprocal(rs[:], s[:])
                    nc.vector.tensor_tensor(
                        probs3[:], exp3[:], rs[:].to_broadcast([128, TC, e]),
                        ALU.mult)
                    nc.sync.dma_start(gpr.ap(), probs3[:])
                    # top-2 over e: m1e = max(exp), zap, m2e = 2nd max
                    m1e = gtp.tile([128, TC], F32)
                    nc.vector.tensor_reduce(m1e[:], exp3[:], AX.X, ALU.max)
                    eq1 = gtp.tile([128, TC, e], F32)
                    nc.vector.tensor_tensor(
                        eq1[:], exp3[:], m1e[:].to_broadcast([128, TC, e]),
                        ALU.is_ge)
                    t1 = gtp.tile([128, TC, e], F32)
                    nc.vector.tensor_tensor(t1[:], eq1[:], exp3[:], ALU.mult)
                    nc.vector.tensor_tensor(t1[:], exp3[:], t1[:],
                                            ALU.subtract)
                    m2e = gtp.tile([128, TC], F32)
                    nc.vector.tensor_reduce(m2e[:], t1[:], AX.X, ALU.max)
                    # top-2 mask, this core's expert, renormalize:
                    # wsel = exp * (exp >= m2e) . esel / (1 + m2e)
                    ge2 = gtp.tile([128, TC, e], F32)
                    nc.vector.tensor_tensor(
                        ge2[:], exp3[:], m2e[:].to_broadcast([128, TC, e]),
                        ALU.is_ge)
                    nc.vector.tensor_tensor(ge2[:], ge2[:], exp3[:], ALU.mult)
                    nc.vector.tensor_tensor(
                        ge2[:], ge2[:],
                        esel_sb[:, None, :].to_broadcast([128, TC, e]),
                        ALU.mult)
                    nc.vector.tensor_reduce(wsel[:], ge2[:], AX.X, ALU.add)
                    m2p1 = gtp.tile([128, TC], F32)
                    nc.vector.tensor_scalar_add(m2p1[:], m2e[:], 1.0)
                    winv = gtp.tile([128, TC], F32)
                    nc.vector.reciprocal(winv[:], m2p1[:])
                    nc.vector.tensor_tensor(wsel[:], wsel[:], winv[:],
                                            ALU.mult)

            def emit_mm2(tb, htb):
                """y rows for block tb: y = ((h @ W2^T) + b2) * wsel."""
                for tcb in range(TCB):
                    tg = tb * TCB + tcb
                    ps2 = ps2pool.tile([128, d], F32)
                    dw = min(512, d)
                    for fc in range(FC):
                        lhsT = htb[:, fc, tcb * 128:(tcb + 1) * 128]
                        for dh in range(d // dw):
                            nc.tensor.matmul(
                                ps2[:, dh * dw:(dh + 1) * dw],
                                lhsT,
                                w2sb[:, fc, dh * dw:(dh + 1) * dw],
                                start=(fc == 0), stop=(fc == FC - 1))
                    yb = ypool.tile([128, d], F32)
                    nc.vector.tensor_add(yb[:], ps2[:], b2_sb[:])
                    nc.vector.tensor_scalar_mul(yb[:], yb[:],
                                                wsel[:, tg:tg + 1])
                    nc.sync.dma_start(
                        y.ap()[tg * 128:(tg + 1) * 128, :], yb[:])

            htb0 = emit_mm1(0)
            emit_gate()
            emit_mm2(0, htb0)
            for tb in range(1, NTB):
                htb = emit_mm1(tb)
                emit_mm2(tb, htb)

    nc.compile()
    return nc


def build_nc_sparse(n_tok=N_TOKENS, d=D_MODEL, f=D_FF, e=N_EXPERTS,
                    cap=1152, act=AF.Gelu, stage=5):
    """Sparse expert-parallel: each core gathers only the tokens routed to
    its expert (top-2 of the gate), computes the FFN on those, and writes
    compacted rows + their token ids; host scatter-adds."""
    import concourse.bass_isa as bass_isa  # noqa: F401
    I32 = mybir.dt.int32
    I16 = mybir.dt.int16
    U32 = mybir.dt.uint32

    KC = d // 128
    TC = n_tok // 128
    FC = f // 128
    NW = n_tok // 16        # wrapped free size for sparse_gather input
    CW = cap // 16          # wrapped free size for compacted lists
    CPC = cap // 128        # 128-token chunks of capacity
    NTG = n_tok // 512      # gate t-groups

    assert cap % 128 == 0 and CW <= 512

    nc = bacc.Bacc("TRN2", target_bir_lowering=False, debug=False,
                   num_devices=N_CORES)

    F16 = mybir.dt.float16
    xT16 = nc.dram_tensor("xT16", [d, n_tok], F16, kind="ExternalInput")
    xrow = nc.dram_tensor("xrow", [n_tok, d], BF16, kind="ExternalInput")
    gwThi = nc.dram_tensor("gwThi", [d, e], BF16, kind="ExternalInput")
    gwTlo = nc.dram_tensor("gwTlo", [d, e], BF16, kind="ExternalInput")
    w1T = nc.dram_tensor("w1T", [d, f], BF16, kind="ExternalInput")
    w2T = nc.dram_tensor("w2T", [f, d], BF16, kind="ExternalInput")
    b1r = nc.dram_tensor("b1r", [128, FC], F32, kind="ExternalInput")
    b2big = nc.dram_tensor("b2big", [128, d], F32, kind="ExternalInput")
    esel = nc.dram_tensor("esel", [128, e], F32, kind="ExternalInput")
    ids_in = nc.dram_tensor("ids_in", [16, NW], F32, kind="ExternalInput")
    yg = nc.dram_tensor("yg", [cap, d], F32, kind="ExternalOutput")
    ids_out = nc.dram_tensor("ids_out", [16, CW], I32, kind="ExternalOutput")
    cnt_out = nc.dram_tensor("cnt_out", [1, 1], U32, kind="ExternalOutput")
    gpr = nc.dram_tensor("gp_raw", [128, TC * e], F32, kind="ExternalOutput")

    xT16_r = xT16.ap().rearrange("(kc p) t -> p kc t", p=128)
    w1T_r = w1T.ap().rearrange("(kc p) ff -> p kc ff", p=128)
    w2T_r = w2T.ap().rearrange("(fc p) dd -> p fc dd", p=128)
    gwThi_r = gwThi.ap().rearrange("(kc p) ee -> p kc ee", p=128)
    gwTlo_r = gwTlo.ap().rearrange("(kc p) ee -> p kc ee", p=128)

    from concourse import library_config

    def _emit(tc):
        with (
            tc.tile_pool(name="persist", bufs=1) as persist,
            tc.tile_pool(name="route", bufs=1) as route,
        ):
            gwhi_sb = persist.tile([128, KC, e], BF16)
            nc.sync.dma_start(gwhi_sb[:], gwThi_r)
            gwlo_sb = persist.tile([128, KC, e], BF16)
            nc.sync.dma_start(gwlo_sb[:], gwTlo_r)
            b1_sb = persist.tile([128, FC], F32)
            nc.sync.dma_start(b1_sb[:], b1r.ap())
            b2_sb = persist.tile([128, d], F32)
            nc.sync.dma_start(b2_sb[:], b2big.ap())
            esel_sb = persist.tile([128, e], F32)
            nc.sync.dma_start(esel_sb[:], esel.ap())
            w2sb = persist.tile([128, FC, d], BF16)
            exp3 = persist.tile([128, TC, e], F32)
            probs3 = persist.tile([128, TC, e], F32)
            wsel = persist.tile([128, TC], F32)

            # ---------------- gate ----------------
            # processed per 512-token group; the softmax/top-2 chain for
            # group g runs on DVE/ACT while the PE computes group g+1
            with (
                tc.tile_pool(name="gx", bufs=3) as gxp,
                tc.tile_pool(name="gps", bufs=4, space="PSUM") as gpp,
                tc.tile_pool(name="gtmp", bufs=2) as gtp,
            ):
                for tg in range(NTG):
                    # fp16-x gate: logits = x16@ghi + x16@glo. gw must stay
                    # a bf16 hi/lo pair (bf16-only gw injects ~2e-3 logit
                    # error); x in fp16 gives ~2e-4, verified to preserve
                    # the top-2 selection with >10x margin. Halves the gate
                    # x DMA vs a bf16 hi/lo pair of x.
                    gxh = gxp.tile([128, KC, 512], F16, tag="gxh")
                    nc.sync.dma_start(
                        gxh[:], xT16_r[:, :, tg * 512:(tg + 1) * 512])
                    for ts4 in range(4):
                        t = tg * 4 + ts4
                        ps = gpp.tile([128, e], F32)
                        for kc in range(KC):
                            xh = gxh[:, kc, ts4 * 128:(ts4 + 1) * 128]
                            nc.tensor.matmul(
                                ps[:], xh, gwhi_sb[:, kc, :],
                                start=(kc == 0), stop=False)
                            nc.tensor.matmul(
                                ps[:], xh, gwlo_sb[:, kc, :],
                                start=False, stop=(kc == KC - 1))
                        nc.vector.tensor_copy(exp3[:, t, :], ps[:])
                    # softmax + top-2 for this group's token slice
                    TG4 = 4
                    sl = slice(tg * TG4, (tg + 1) * TG4)
                    E3 = exp3[:, sl, :]
                    P3 = probs3[:, sl, :]
                    mx = gtp.tile([128, TG4], F32, tag="mx")
                    nc.vector.tensor_reduce(mx[:], E3, AX.X, ALU.max)
                    nc.vector.tensor_tensor(
                        E3, E3, mx[:].to_broadcast([128, TG4, e]),
                        ALU.subtract)
                    nc.scalar.activation(E3, E3, AF.Exp)
                    s = gtp.tile([128, TG4], F32, tag="s")
                    nc.vector.tensor_reduce(s[:], E3, AX.X, ALU.add)
                    rs = gtp.tile([128, TG4], F32, tag="rs")
                    nc.vector.reciprocal(rs[:], s[:])
                    nc.vector.tensor_tensor(
                        P3, E3, rs[:].to_broadcast([128, TG4, e]), ALU.mult)
                    m1e = gtp.tile([128, TG4], F32, tag="m1e")
                    nc.vector.tensor_reduce(m1e[:], E3, AX.X, ALU.max)
                    eq1 = gtp.tile([128, TG4, e], F32, tag="eq1")
                    nc.vector.tensor_tensor(
                        eq1[:], E3, m1e[:].to_broadcast([128, TG4, e]),
                        ALU.is_ge)
                    t1 = gtp.tile([128, TG4, e], F32, tag="t1")
                    nc.vector.tensor_tensor(t1[:], eq1[:], E3, ALU.mult)
                    nc.vector.tensor_tensor(t1[:], E3, t1[:], ALU.subtract)
                    m2e = gtp.tile([128, TG4], F32, tag="m2e")
                    nc.vector.tensor_reduce(m2e[:], t1[:], AX.X, ALU.max)
                    ge2 = gtp.tile([128, TG4, e], F32, tag="ge2")
                    nc.vector.tensor_tensor(
                        ge2[:], E3, m2e[:].to_broadcast([128, TG4, e]),
                        ALU.is_ge)
                    nc.vector.tensor_tensor(ge2[:], ge2[:], E3, ALU.mult)
                    nc.vector.tensor_tensor(
                        ge2[:], ge2[:],
                        esel_sb[:, None, :].to_broadcast([128, TG4, e]),
                        ALU.mult)
                    nc.vector.tensor_reduce(wsel[:, sl], ge2[:], AX.X,
                                            ALU.add)
                    m2p1 = gtp.tile([128, TG4], F32, tag="m2p1")
                    nc.vector.tensor_scalar_add(m2p1[:], m2e[:], 1.0)
                    winv = gtp.tile([128, TG4], F32, tag="winv")
                    nc.vector.reciprocal(winv[:], m2p1[:])
                    nc.vector.tensor_tensor(wsel[:, sl], wsel[:, sl],
                                            winv[:], ALU.mult)
                nc.sync.dma_start(gpr.ap(), probs3[:])
            if stage <= 1:
                return

            # ---------------- routing: compact this expert's tokens --------
            # wrapped layout: token t at [t % 16, t // 16]
            wselw = route.tile([16, NW], F32)
            wselw3 = wselw[:].rearrange("q (ft a) -> q ft a", a=8)
            for a in range(8):
                # partition-base-16a access: engines need quadrant-aligned
                # partition starts, so shuffle via DMA
                nc.sync.dma_start(
                    wselw3[:, :, a], wsel[16 * a:16 * (a + 1), :])
            # wrapped token ids [16, NW] as f32, provided by the host (a
            # static iota; avoids a GpSimd library switch for InstIota)
            ids_f = route.tile([16, NW], F32)
            nc.sync.dma_start(ids_f[:], ids_in.ap())
            maskw = route.tile([16, NW], F32)
            nc.vector.tensor_scalar(maskw[:], wselw[:], 0.0, None,
                                    op0=ALU.is_gt)
            # sel_id = (id+1)*mask - 1  (id where selected, -1 elsewhere)
            sel_id = route.tile([16, NW], F32)
            nc.vector.tensor_scalar(sel_id[:], ids_f[:], 1.0, None,
                                    op0=ALU.add)
            nc.vector.tensor_tensor(sel_id[:], sel_id[:], maskw[:], ALU.mult)
            nc.vector.tensor_scalar(sel_id[:], sel_id[:], 1.0, None,
                                    op0=ALU.subtract)
            # sel_w = w + (mask-1)  (w>0 where selected, -1 elsewhere)
            sel_w = route.tile([16, NW], F32)
            m1t = route.tile([16, NW], F32)
            nc.vector.tensor_scalar(m1t[:], maskw[:], 1.0, None,
                                    op0=ALU.subtract)
            nc.vector.tensor_add(sel_w[:], wselw[:], m1t[:])

            ids_c = route.tile([16, CW], F32)
            cnt = route.tile([1, 1], U32)
            nc.vector.memset(ids_c[:], -1.0)
            nc.gpsimd.sparse_gather(ids_c[:], sel_id[:], num_found=cnt[:])
            w_c = route.tile([16, CW], F32)
            cnt2 = route.tile([1, 1], U32)
            nc.vector.memset(w_c[:], -1.0)
            nc.gpsimd.sparse_gather(w_c[:], sel_w[:], num_found=cnt2[:])
            nc.sync.dma_start(cnt_out.ap(), cnt[:])
            # On HW the tail beyond num_found is uninitialized garbage (the
            # sim fills -1): force tail slots to 0 by position, overwriting
            # whatever junk is there (ids -> token 0, weights -> 0).
            # cnt is broadcast to 16 partitions with a K=1 ones matmul on the
            # (idle) PE instead of gpsimd.partition_broadcast — keeps the
            # GpSimd op sequence inside a single ucode library.
            cnt_f = route.tile([1, 1], F32)
            nc.vector.tensor_copy(cnt_f[:], cnt[:])
            ones16 = route.tile([1, 16], F32)
            nc.vector.memset(ones16[:], 1.0)
            with tc.tile_pool(name="cps", bufs=1, space="PSUM") as cpsp:
                cps = cpsp.tile([16, 1], F32)
                nc.tensor.matmul(cps[:], ones16[:], cnt_f[:],
                                 start=True, stop=True)
                cntb = route.tile([16, 1], F32)
                nc.vector.tensor_copy(cntb[:], cps[:])
            notkeep = route.tile([16, CW], U32)
            nc.vector.tensor_tensor(notkeep[:], ids_f[:, :CW],
                                    cntb[:].to_broadcast([16, CW]), ALU.is_ge)
            zeros16 = route.tile([16, CW], F32)
            nc.vector.memset(zeros16[:], 0.0)
            nc.vector.copy_predicated(ids_c[:], notkeep[:], zeros16[:])
            nc.vector.copy_predicated(w_c[:], notkeep[:], zeros16[:])
            ids32 = route.tile([16, CW], I32)
            nc.vector.tensor_copy(ids32[:], ids_c[:])
            nc.sync.dma_start(ids_out.ap(), ids32[:])
            ids16 = route.tile([16, CW], I16)
            nc.vector.tensor_copy(ids16[:], ids_c[:])
            idx128 = route.tile([128, CW], I16)
            for a in range(8):
                nc.sync.dma_start(idx128[16 * a:16 * (a + 1), :], ids16[:])
            # per-128-chunk gate weights: wpart[16a+q, c] = w_c[q, 8c+a]
            wpart = route.tile([128, CPC], F32)
            w_c3 = w_c[:].rearrange("q (c a) -> q c a", a=8)
            for a in range(8):
                nc.sync.dma_start(wpart[16 * a:16 * (a + 1), :],
                                  w_c3[:, :, a])

            if stage <= 2:
                return

            # ---------------- gather x rows ----------------
            with (
                tc.tile_pool(name="xg", bufs=1) as xgp,
                tc.tile_pool(name="hg", bufs=1) as hgp,
                tc.tile_pool(name="w1s", bufs=2) as w1pool,
                tc.tile_pool(name="ps1", bufs=2, space="PSUM") as ps1pool,
                tc.tile_pool(name="ps2", bufs=2, space="PSUM") as ps2pool,
                tc.tile_pool(name="yev", bufs=3) as ypool,
            ):
                # chunks of the capacity: <=512 wide for the matmul free-dim
                # limit AND for dma_gather (one gather's s2m descriptor
                # count must fit the 128-entry SWDGE ring -> <=512 idxs)
                # smallest chunk first: its descriptor generation + transfer
                # finish quickest, so mm1's first matmuls start sooner
                widths = [256, 512, 512] if cap == 1280 else [256, 512, 384]
                if cap not in (1280, 1152):
                    widths = []
                    off = 0
                    while off < cap:
                        widths.append(min(512, cap - off))
                        off += widths[-1]
                chunks = []
                off = 0
                for cwid in widths:
                    chunks.append((off, cwid))
                    off += cwid
                assert off == cap

                xg_tiles = {}
                for (off, cwid) in chunks:
                    xgc = xgp.tile([128, KC, cwid], BF16, tag=f"xg{off}")
                    nc.gpsimd.dma_gather(
                        xgc[:], xrow.ap(),
                        idx128[:, off // 16:(off + cwid) // 16],
                        num_idxs=cwid, num_idxs_reg=cwid, elem_size=d,
                        transpose=True)
                    xg_tiles[off] = xgc
                # w2 (8MB) is first needed by mm2. The GpSimd ucode-library
                # switch before the gathers drains ALL in-flight SWDGE DMA,
                # so an early-running w2 transfer would stall it ~22us.
                # Write a dummy sliver of w2sb from the last gather's output
                # first: the WAW dependency forces the w2 DMA after the
                # gathers have issued.
                last_xg = xg_tiles[chunks[-1][0]]
                nc.vector.tensor_copy(w2sb[:, 0, 0:2], last_xg[:, 0, 0:2])
                nc.sync.dma_start(w2sb[:], w2T_r)
                if stage <= 3:
                    ytmp = ypool.tile([128, d], F32)
                    nc.vector.tensor_copy(ytmp[:, 0:cap // 4],
                                          xg_tiles[0][:, 0, 0:cap // 4])
                    nc.sync.dma_start(yg.ap()[0:128, :], ytmp[:])
                    return

                hg = hgp.tile([128, FC, cap], BF16)
                FG = min(512, f)
                for fg in range(f // FG):
                    w1g = w1pool.tile([128, KC, FG], BF16)
                    nc.sync.dma_start(
                        w1g[:], w1T_r[:, :, fg * FG:(fg + 1) * FG])
                    for fs in range(FG // 128):
                        fc = fg * (FG // 128) + fs
                        for (off, cwid) in chunks:
                            ps = ps1pool.tile([128, 512], F32)
                            for kc in range(KC):
                                nc.tensor.matmul(
                                    ps[:, :cwid],
                                    w1g[:, kc, fs * 128:(fs + 1) * 128],
                                    xg_tiles[off][:, kc, :],
                                    start=(kc == 0), stop=(kc == KC - 1))
                            nc.scalar.activation(
                                hg[:, fc, off:off + cwid], ps[:, :cwid],
                                act, bias=b1_sb[:, fc:fc + 1], scale=1.0)

                if stage <= 4:
                    ytmp = ypool.tile([128, d], F32)
                    nc.vector.tensor_copy(ytmp[:, 0:cap // 4],
                                          hg[:, 0, 0:cap // 4])
                    nc.sync.dma_start(yg.ap()[0:128, :], ytmp[:])
                    return

                dw = min(512, d)
                for tcb in range(CPC):
                    ps2 = ps2pool.tile([128, d], F32)
                    for fc in range(FC):
                        lhsT = hg[:, fc, tcb * 128:(tcb + 1) * 128]
                        for dh in range(d // dw):
                            nc.tensor.matmul(
                                ps2[:, dh * dw:(dh + 1) * dw],
                                lhsT,
                                w2sb[:, fc, dh * dw:(dh + 1) * dw],
                                start=(fc == 0), stop=(fc == FC - 1))
                    yb = ypool.tile([128, d], F32)
                    nc.vector.tensor_add(yb[:], ps2[:], b2_sb[:])
                    nc.vector.tensor_scalar_mul(yb[:], yb[:],
                                                wpart[:, tcb:tcb + 1])
                    nc.sync.dma_start(
                        yg.ap()[tcb * 128:(tcb + 1) * 128, :], yb[:])

    with tile.TileContext(nc) as tc:
        _emit(tc)
    nc.compile()
    return nc


_NC_CACHE = {}


def _get_nc():
    key = (N_TOKENS, D_MODEL, D_FF, N_EXPERTS, T_BLK)
    if key not in _NC_CACHE:
        _NC_CACHE[key] = build_nc(*key)
    return _NC_CACHE[key]


def prep_core_inputs(x, gate_w, w1, b1, w2, b2, n_cores=N_CORES):
    """Host-side sharding: per-core input dicts (expert-parallel)."""
    bf16 = ml_dtypes.bfloat16
    n = x.shape[0] * x.shape[1]
    d = x.shape[2]
    f = w1.shape[1]
    xT32 = np.ascontiguousarray(x.reshape(n, d).T.astype(np.float32))
    xTbf = np.ascontiguousarray(xT32.astype(bf16))
    gwT = np.ascontiguousarray(gate_w.T.astype(np.float32))
    in_maps = []
    for c in range(n_cores):
        e = c % N_EXPERTS
        onehot = np.zeros((128, N_EXPERTS), np.float32)
        onehot[:, e] = 1.0
        in_maps.append({
            "xT32": xT32,
            "xTbf": xTbf,
            "gwT": gwT,
            "w1T": np.ascontiguousarray(w1[e].T.astype(bf16)),
            "w2T": np.ascontiguousarray(w2[e].T.astype(bf16)),
            "b1r": np.ascontiguousarray(
                b1[e].reshape(f // 128, 128).T.astype(np.float32)),
            "b2big": np.ascontiguousarray(
                np.broadcast_to(b2[e].astype(np.float32), (128, d))),
            "esel": onehot,
        })
    return in_maps


CAP = 1152


def prep_core_inputs_sparse(x, gate_w, w1, b1, w2, b2, n_cores=N_CORES):
    bf16 = ml_dtypes.bfloat16
    n = x.shape[0] * x.shape[1]
    d = x.shape[2]
    f = w1.shape[1]
    xf = np.ascontiguousarray(x.reshape(n, d).astype(np.float32))
    xT32 = np.ascontiguousarray(xf.T)
    xT16 = xT32.astype(np.float16)
    gwT = np.ascontiguousarray(gate_w.T.astype(np.float32))
    gwThi = gwT.astype(bf16)
    gwTlo = (gwT - gwThi.astype(np.float32)).astype(bf16)
    xrow = np.ascontiguousarray(xf.astype(bf16))
    # wrapped token ids: token t at [t % 16, t // 16]
    ids_in = np.ascontiguousarray(
        np.arange(n, dtype=np.float32).reshape(n // 16, 16).T)
    in_maps = []
    for c in range(n_cores):
        e = c % N_EXPERTS
        onehot = np.zeros((128, N_EXPERTS), np.float32)
        onehot[:, e] = 1.0
        in_maps.append({
            "xT16": xT16,
            "xrow": xrow,
            "gwThi": gwThi,
            "gwTlo": gwTlo,
            "ids_in": ids_in,
            "w1T": np.ascontiguousarray(w1[e].T.astype(bf16)),
            "w2T": np.ascontiguousarray(w2[e].T.astype(bf16)),
            "b1r": np.ascontiguousarray(
                b1[e].reshape(f // 128, 128).T.astype(np.float32)),
            "b2big": np.ascontiguousarray(
                np.broadcast_to(b2[e].astype(np.float32), (128, d))),
            "esel": onehot,
        })
    return in_maps


def _combine_sparse(res, B, S, d, n):
    out = np.zeros((n, d), np.float32)
    overflow = False
    for c in range(N_CORES):
        r = res.results[c]
        cnt = int(r["cnt_out"][0, 0])
        if cnt > CAP:
            overflow = True
        ids = r["ids_out"].T.ravel()
        np.add.at(out, ids, r["yg"])
    gp_raw = res.results[0]["gp_raw"]
    TC = n // 128
    gp = (gp_raw.reshape(128, TC, N_EXPERTS)
          .transpose(1, 0, 2).reshape(B, S, N_EXPERTS))
    return out.reshape(B, S, d), gp, overflow


USE_SPARSE = True


def kernel(x, gate_w, w1, b1, w2, b2, trace=False):
    B, S, d = x.shape
    n = B * S
    if USE_SPARSE:
        key = ("sparse", n, d, D_FF, N_EXPERTS, CAP)
        if key not in _NC_CACHE:
            _NC_CACHE[key] = build_nc_sparse(n, d, D_FF, N_EXPERTS, CAP)
        nc = _NC_CACHE[key]
        in_maps = prep_core_inputs_sparse(x, gate_w, w1, b1, w2, b2)
        res = bass_utils.run_bass_kernel_spmd(
            nc, in_maps, core_ids=list(range(N_CORES)), trace=trace)
        out, gp, overflow = _combine_sparse(res, B, S, d, n)
        if trace:
            kernel.last_results = res
        if not overflow:
            return out, gp
        # capacity overflow (should not happen): fall through to dense
    nc = _get_nc()
    in_maps = prep_core_inputs(x, gate_w, w1, b1, w2, b2)
    res = bass_utils.run_bass_kernel_spmd(
        nc, in_maps, core_ids=list(range(N_CORES)), trace=trace)
    out = res.results[0]["y"].astype(np.float64)
    for c in range(1, N_CORES):
        out = out + res.results[c]["y"]
    out = out.astype(np.float32).reshape(B, S, d)
    gp_raw = res.results[0]["gp_raw"]
    TC = n // 128
    gp = (gp_raw.reshape(128, TC, N_EXPERTS)
          .transpose(1, 0, 2).reshape(B, S, N_EXPERTS))
    if trace:
        kernel.last_results = res
    return out, gp
